# revision 7
# baseline (speedup 1.0000x reference)
"""ExplaiNN Trainium2 kernel — 8-core SPMD, batch-sharded (32 rows/core).

Restructured from the 154.5us baseline (cost-model findings: DVE 106us busy
on fp32 reduce-max pooling + FC2; Pool engine 65us of SWDGE descriptor
generation; PE 41% busy). Now 119.6us with rel err 3.1e-3 (was 1.5e-2).

  dtype: fp16 matmul operands everywhere (vs fp32r/bf16) — halves DMA and
         keeps full-rate PE matmuls at any stream width, with better
         precision than bf16.
  conv:  X-stationary matmuls as before; per row j0-2 land in a 3-bank PSUM
         tile, j3-6 in a 4-bank tile. B-blocks (tail 13 pool windows x 8
         rows) run FIRST so their pexpB2 repack DMA chain hides under the
         A-row stream.
  pool:  only DVE and ACT can read PSUM (the Pool engine is DMA-only and a
         TensorTensor allows at most one PSUM input — hw limits). DVE
         reduce_max eats j0-2; one batched ACT copy moves j3-6 to fp16
         SBUF; DVE merges the 5 partials as a pair-max tree in fp16 2x
         mode, batched over 4 rows; batched ACT exp writes pexp. Merges are
         emitted one 4-row batch late and exps a batch later so they never
         park in the 4-deep wait queues and stall their sequencer (convoy
         avoidance); weight prefetch rides the otherwise-idle Pool queue.
  FC1:   weight-stationary per unit (ldweights w1 [128,100], stream pexp
         [128,32]) -> h lands f-major [100f, 32b] in PSUM, 16 units/bank,
         triple-buffered; ACT relu -> hrelu fp16 (+const-1 row 100 carrying
         the FC2 bias). K-tail weights w1b packed 3 units per partition
         group at offsets 0/32/64 (matmul base-partition constraint), with
         pexpB2 replicated to those offsets.
  FC2:   per 4-unit group one PE pair: lhsT = hrelu [101, 4ux32b], rhs =
         w2 [101, 4] -> block-diagonal strips of zps [128, 300].
  head:  per strip: DVE relu, mul, reduce; partition mixdown via Emat
         matmul; ACT sigmoid; DMA out.
"""

import numpy as np
from contextlib import ExitStack

import concourse.bass as bass
import concourse.bacc as bacc
import concourse.mybir as mybir
import concourse.tile as tile
from concourse.bass_utils import run_bass_kernel_spmd

dt = mybir.dt

U, K, POOL, STRIDE, FC = 300, 19, 7, 7, 100
B, L, D = 256, 1000, 4
P = 140                     # pooled positions per row
EPS = 1e-5
NCORES = 8
BS = B // NCORES            # 32 rows per core
KD = K * D                  # 76 contraction
PA = 127                    # pool windows in the A-chunk (+1 const row = 128)
PB = P - PA                 # 13 windows in the B-chunk
CH = [(0, 300)]              # single conv pass
NG = U // 4                 # FC2 4-unit groups

_COMPILED = None


def _build():
    nc = bacc.Bacc("TRN2", target_bir_lowering=False, debug=False,
                   num_devices=NCORES)

    f16, f32 = dt.float16, dt.float32
    AF = mybir.ActivationFunctionType
    ALU = mybir.AluOpType

    xcol_d = nc.dram_tensor("xcol", [KD, BS, 980], f16, kind="ExternalInput").ap()
    xcolb_d = nc.dram_tensor("xcolb", [KD, 4, 7, 104], f16, kind="ExternalInput").ap()
    wc_d = nc.dram_tensor("wc", [KD, U], f16, kind="ExternalInput").ap()
    w1a_d = nc.dram_tensor("w1a", [128, U, FC], f16, kind="ExternalInput").ap()
    w1b_d = nc.dram_tensor("w1b", [64 + PB, FC, FC], f16, kind="ExternalInput").ap()
    w2_d = nc.dram_tensor("w2s", [FC + 1, NG, 4], f16, kind="ExternalInput").ap()
    wout_d = nc.dram_tensor("woute", [128, NG], f32, kind="ExternalInput").ap()
    E_d = nc.dram_tensor("Emat", [128, BS], f32, kind="ExternalInput").ap()
    ones_d = nc.dram_tensor("onesrow", [1, BS * U], f16, kind="ExternalInput").ap()
    bout_d = nc.dram_tensor("bout", [1, 1], f32, kind="ExternalInput").ap()
    out_d = nc.dram_tensor("out", [1, BS], f32, kind="ExternalOutput").ap()

    with ExitStack() as ctx:
        tc = ctx.enter_context(tile.TileContext(nc))
        consts = ctx.enter_context(tc.tile_pool(name="consts", bufs=1))

        wc = consts.tile([KD, U], f16)
        xcolb = consts.tile([KD, 4, 7, 104], f16)
        w1a = consts.tile([128, U, FC], f16)
        w1b = consts.tile([64 + PB, FC, FC], f16)  # 3 units at partition 0/32/64
        w2s = consts.tile([FC + 1, NG, 4], f16)
        woute = consts.tile([128, NG], f32)
        Emat = consts.tile([128, BS], f32)
        bout = consts.tile([1, 1], f32)
        pexp = consts.tile([128, BS, U], f16)       # [p(127)+const, b, u]
        pexpB = consts.tile([104, 4, U], f16)       # [(rr,pb), blk, u]
        pexpB2 = consts.tile([64 + PB, BS, U], f16)  # [pb, b, u] replicated @0/32/64

        nc.sync.dma_start(wc[:], wc_d[:])
        nc.sync.dma_start(xcolb[:], xcolb_d[:])
        nc.sync.dma_start(pexp[127:128, :, :].rearrange("p b u -> p (b u)"),
                            ones_d[:])

        # ---------------- conv + pool + exp ---------------------------------
        # Only DVE and ACT can read PSUM (Pool engine is DMA-only; a
        # TensorTensor may use at most one PSUM input). Per row: j0-2 land in
        # a 3-bank tile -> DVE reduce_max; j3-6 land in a 4-bank tile -> one
        # batched ACT copy to fp16 SBUF. The 5 partials merge on DVE as a
        # pair-max tree in fp16 2x mode, batched over 4 rows. Merges are
        # emitted one 4-row batch late and exps two batches late so neither
        # ever parks in the 4-deep wait queues and stalls its sequencer.
        pm = consts.tile([128, 36, U], f16)         # merged pool maxes
        with tc.tile_pool(name="xslab", bufs=3) as xpool, \
             tc.tile_pool(name="redps", bufs=1, space="PSUM") as rpsum, \
             tc.tile_pool(name="cpys", bufs=1, space="PSUM") as cpsum, \
             tc.tile_pool(name="qpool", bufs=2) as qpool, \
             tc.tile_pool(name="mpool", bufs=1) as mpool:


            for sq in range(4):
                nc.gpsimd.dma_start(w1a[:, 75 * sq:75 * sq + 75, :],
                                    w1a_d[:, 75 * sq:75 * sq + 75, :])
            nc.gpsimd.dma_start(w1b[:], w1b_d[:])
            nc.gpsimd.dma_start(w2s[:], w2_d[:])
            nc.gpsimd.dma_start(woute[:], wout_d[:])
            nc.gpsimd.dma_start(Emat[:], E_d[:])
            nc.gpsimd.dma_start(bout[:], bout_d[:])

            def conv_pool(lhs7, m, Q, C, qi):
                t3 = rpsum.tile([128, 3, 512], f32, tag="t3")
                for j in range(3):
                    nc.tensor.matmul(t3[0:m, j, 0:U], lhs7[j], wc[:],
                                     start=True, stop=True)
                t4 = cpsum.tile([128, 4, 512], f32, tag="t4")
                for j in range(3, 7):
                    nc.tensor.matmul(t4[0:m, j - 3, 0:U], lhs7[j], wc[:],
                                     start=True, stop=True)
                nc.vector.reduce_max(
                    Q[0:m, qi, :], t3[0:m, :, 0:U].rearrange("p j u -> p u j"),
                    axis=mybir.AxisListType.X)
                nc.scalar.activation(C[0:m, qi, :, :], t4[0:m, 0:4, 0:U], AF.Copy)

            def merges(Q, C, m, s0):
                m2 = mpool.tile([128, 4, 2, 300], f16, tag="m2")
                nc.vector.tensor_max(m2[0:m, :, :, :], C[0:m, :, 0:2, :],
                                     C[0:m, :, 2:4, :])
                m3 = mpool.tile([128, 4, 300], f16, tag="m3")
                nc.vector.tensor_max(m3[0:m, :, :], m2[0:m, :, 0, :],
                                     m2[0:m, :, 1, :])
                nc.vector.tensor_max(pm[0:m, s0:s0 + 4, :], m3[0:m, :, :],
                                     Q[0:m, :, :])

            batches = []     # (Q, C, m, pm-slot, exp destination)
            done_m, done_e = 0, 0

            def drain(upto_m, upto_e):
                nonlocal done_m, done_e
                while done_m < upto_m:
                    Q, C, m, s0, _ = batches[done_m]
                    merges(Q, C, m, s0)
                    done_m += 1
                while done_e < upto_e:
                    _, _, m, s0, dst = batches[done_e]
                    nc.scalar.activation(dst, pm[0:m, s0:s0 + 4, :], AF.Exp)
                    done_e += 1

            # B blocks first so their repack DMAs hide under the A rows
            Q = qpool.tile([128, 4, 300], f16, tag="Q")
            C = qpool.tile([128, 4, 4, 300], f16, tag="C")
            for blk in range(4):
                lhs7 = [xcolb[:, blk, j, :] for j in range(7)]
                conv_pool(lhs7, 8 * PB, Q, C, blk)
            batches.append((Q, C, 8 * PB, 32, pexpB[0:8 * PB, 0:4, :]))
            for sb in range(8):
                slab = xpool.tile([KD, 4, 980], f16, tag="slab")
                nc.sync.dma_start(slab[:], xcol_d[:, 4 * sb:4 * sb + 4, :])
                Q = qpool.tile([128, 4, 300], f16, tag="Q")
                C = qpool.tile([128, 4, 4, 300], f16, tag="C")
                slabr = slab[:].rearrange("q r (p j) -> q r p j", j=7)
                for r in range(4):
                    lhs7 = [slabr[:, r, 0:PA, j] for j in range(7)]
                    conv_pool(lhs7, PA, Q, C, r)
                    if r == 1:
                        drain(sb + 1, sb)
                if sb == 1:
                    # repack pexpB[(rr,pb), blk, u] -> pexpB2[pb, 8*blk+rr, u]
                    for rr in range(8):
                        nc.sync.dma_start(pexpB2[0:PB, rr:BS:8, :],
                                          pexpB[rr * PB:(rr + 1) * PB, :, :])
                    for off in (32, 64):
                        nc.sync.dma_start(pexpB2[off:off + PB, :, :],
                                          pexpB2[0:PB, :, :])
                batches.append((Q, C, PA, 4 * sb,
                                pexp[0:PA, 4 * sb:4 * sb + 4, :]))
            drain(9, 9)

        # ---------------- FC1 + relu + FC2 + head ---------------------------
        fcpool = ctx.enter_context(tc.tile_pool(name="fcsb", bufs=1))
        hrelu = fcpool.tile([FC + 1, U, BS], f16)   # [f+const, u, b]
        nc.sync.dma_start(hrelu[FC:FC + 1, :, :].rearrange("p u b -> p (u b)"),
                            ones_d[:])
        zps_pool = ctx.enter_context(tc.tile_pool(name="zpsp", bufs=1, space="PSUM"))
        zps = zps_pool.tile([128, U], f32)

        with tc.tile_pool(name="fcps", bufs=3, space="PSUM") as fpsum:
            for ci, (c0, w) in enumerate(CH):
                for g in range((w + 15) // 16):
                    u0 = c0 + 16 * g
                    nun = min(16, c0 + w - u0)
                    hps = fpsum.tile([FC, 16, BS], f32, tag="hps")
                    for s in range(nun):
                        u = u0 + s
                        o = hps[0:FC, s, 0:BS]
                        nc.tensor.matmul(o, w1a[:, u, :], pexp[:, :, u],
                                         start=True, stop=False)
                        off = 32 * (u % 3)
                        nc.tensor.matmul(
                            o, w1b[off:off + PB, u // 3, :],
                            pexpB2[off:off + PB, :, u], start=False, stop=True)
                    nc.scalar.activation(hrelu[0:FC, u0:u0 + nun, :],
                                         hps[0:FC, 0:nun, 0:BS], AF.Relu)
                    for k in range(u0 // 4, (u0 + nun) // 4):
                        nc.tensor.matmul(
                            zps[0:128, 4 * k:4 * k + 4],
                            hrelu[0:FC + 1, 4 * k:4 * k + 4, :].rearrange(
                                "f u b -> f (u b)"),
                            w2s[:, k, :], start=True, stop=True)

            # head: per strip n: relu then fused mul+reduce over u
            part = fcpool.tile([128, 1], f32)
            zr = fcpool.tile([128, NG], f32)
            prod = fcpool.tile([128, NG], f32)
            for n in range(4):
                sl = slice(32 * n, 32 * n + 32)
                nc.vector.tensor_scalar_max(zr[sl, :], zps[sl, n:U:4], 0.0)
                nc.vector.tensor_mul(prod[sl, :], zr[sl, :], woute[sl, :])
                nc.vector.tensor_reduce(part[sl, 0:1], prod[sl, :],
                                        axis=mybir.AxisListType.X, op=ALU.add)
            with tc.tile_pool(name="headps", bufs=1, space="PSUM") as hpsum:
                zf = hpsum.tile([1, BS], f32, tag="zf")
                nc.tensor.matmul(zf[0:1, :], part[:], Emat[:], start=True, stop=True)
                osb = fcpool.tile([1, BS], f32)
                nc.scalar.activation(osb[:], zf[0:1, :], AF.Sigmoid, bias=bout[0:1, :])
                nc.sync.dma_start(out_d[:], osb[:])

    nc.compile()
    return nc


def _prep_weights(i):
    """Host-side BN folding + layout. numpy fp32 math -> fp16 payloads."""
    f = lambda a: np.asarray(a, np.float32)
    w_conv, b_conv = f(i["w_conv"]), f(i["b_conv"])
    g1, be1, m1, v1 = f(i["g1"]), f(i["be1"]), f(i["m1"]), f(i["v1"])
    w_fc1, b_fc1 = f(i["w_fc1"]), f(i["b_fc1"])
    g2, be2, m2, v2 = f(i["g2"]), f(i["be2"]), f(i["m2"]), f(i["v2"])
    w_fc2, b_fc2 = f(i["w_fc2"]), f(i["b_fc2"])
    g3, be3, m3, v3 = f(i["g3"]), f(i["be3"]), f(i["m3"]), f(i["v3"])
    w_out, b_out = f(i["w_out"]), f(i["b_out"])

    s1 = g1 / np.sqrt(v1 + EPS)
    t1 = be1 - m1 * s1
    s2 = g2 / np.sqrt(v2 + EPS)
    b1pp = (b_fc1 - m2) * s2 + be2
    s3 = g3 / np.sqrt(v3 + EPS)
    w2pp = w_fc2 * s3[:, None]
    b2pp = (b_fc2 - m3) * s3 + be3

    # conv weights with BN1 scale folded; contraction index q = k*D + d
    Wc = np.ascontiguousarray(
        (w_conv * s1[:, None, None]).transpose(2, 1, 0).reshape(KD, U))
    # FC1 with BN2 scale and exp(t1 + s1*b_conv) folded
    gexp = np.exp(t1 + s1 * b_conv)
    w1pp = (w_fc1 * s2[:, :, None] * gexp[:, None, None]).transpose(2, 0, 1)  # (P,U,FC)
    w1a = np.empty((128, U, FC), np.float32)
    w1a[:PA] = w1pp[:PA]
    w1a[127] = b1pp                      # bias rides the const-1 pexp row
    # w1b: 3 units per partition group at offsets 0/32/64: [32*(u%3)+pb, u//3, f]
    w1b = np.zeros((64 + PB, FC, FC), np.float32)
    for u in range(U):
        w1b[32 * (u % 3):32 * (u % 3) + PB, u // 3] = w1pp[PA:P, u]

    # FC2 weights f-major with bias row: w2s[f, k, n] = w2pp[4k+n, f]
    w2s = np.empty((FC + 1, NG, 4), np.float32)
    w2s[:FC] = w2pp.T.reshape(FC, NG, 4)
    w2s[FC] = b2pp.reshape(NG, 4)

    # head: strip n rows 32n..32n+32 hold w_out[n::4]
    woute = np.zeros((128, NG), np.float32)
    for n in range(4):
        woute[32 * n:32 * n + 32] = w_out[n::4, 0][None]
    Em = np.zeros((128, BS), np.float32)
    for n in range(4):
        Em[32 * n:32 * n + 32] = np.eye(BS, dtype=np.float32)

    h16 = lambda a: np.asarray(a, np.float16)
    return {
        "wc": h16(Wc), "w1a": h16(w1a), "w1b": h16(w1b), "w2s": h16(w2s),
        "woute": woute, "Emat": Em,
        "onesrow": np.ones((1, BS * U), np.float16),
        "bout": np.asarray(b_out, np.float32).reshape(1, 1),
    }


def kernel(**inputs) -> np.ndarray:
    global _COMPILED
    if _COMPILED is None:
        _COMPILED = _build()
    nc = _COMPILED

    wmap = _prep_weights(inputs)
    x = np.asarray(inputs["input_seq"], np.float32)   # (256, 1000, 4)
    win = np.lib.stride_tricks.sliding_window_view(x, K, axis=1)  # (B, 982, D, K)
    in_maps = []
    for c in range(NCORES):
        xs = win[c * BS:(c + 1) * BS, :980]           # (32, 980, 4, 19)
        xcol = np.ascontiguousarray(
            xs.transpose(3, 2, 0, 1).astype(np.float16)).reshape(KD, BS, 980)
        tail = xcol[:, :, 7 * PA:].reshape(KD, 4, 8, PB, 7)
        xcolb = np.ascontiguousarray(tail.transpose(0, 1, 4, 2, 3)).reshape(KD, 4, 7, 104)
        in_maps.append({"xcol": xcol, "xcolb": xcolb, **wmap})

    res = run_bass_kernel_spmd(nc, in_maps, list(range(NCORES)))
    out = np.empty((B, 1), np.float32)
    for c in range(NCORES):
        out[c * BS:(c + 1) * BS, 0] = res.results[c]["out"][0]
    return out


# revision 14
# speedup vs baseline: 1.0553x; 1.0553x over previous
"""ExplaiNN Trainium2 kernel — 8-core SPMD, batch-sharded (32 rows/core).

Restructured from the 154.5us baseline (cost-model findings: DVE 106us busy
on fp32 reduce-max pooling + FC2; Pool engine 65us of SWDGE descriptor
generation; PE 41% busy). Now 119.6us with rel err 3.1e-3 (was 1.5e-2).

  dtype: fp16 matmul operands everywhere (vs fp32r/bf16) — halves DMA and
         keeps full-rate PE matmuls at any stream width, with better
         precision than bf16.
  conv:  X-stationary matmuls as before; per row j0-2 land in a 3-bank PSUM
         tile, j3-6 in a 4-bank tile. B-blocks (tail 13 pool windows x 8
         rows) run FIRST so their pexpB2 repack DMA chain hides under the
         A-row stream.
  pool:  only DVE and ACT can read PSUM (the Pool engine is DMA-only and a
         TensorTensor allows at most one PSUM input — hw limits). DVE
         reduce_max eats j0-2; one batched ACT copy moves j3-6 to fp16
         SBUF; DVE merges the 5 partials as a pair-max tree in fp16 2x
         mode, batched over 4 rows; batched ACT exp writes pexp. Merges are
         emitted one 4-row batch late and exps a batch later so they never
         park in the 4-deep wait queues and stall their sequencer (convoy
         avoidance); weight prefetch rides the otherwise-idle Pool queue.
  FC1:   weight-stationary per unit (ldweights w1 [128,100], stream pexp
         [128,32]) -> h lands f-major [100f, 32b] in PSUM, 16 units/bank,
         triple-buffered; ACT relu -> hrelu fp16 (+const-1 row 100 carrying
         the FC2 bias). K-tail weights w1b packed 3 units per partition
         group at offsets 0/32/64 (matmul base-partition constraint), with
         pexpB2 replicated to those offsets.
  FC2:   per 4-unit group one PE pair: lhsT = hrelu [101, 4ux32b], rhs =
         w2 [101, 4] -> block-diagonal strips of zps [128, 300].
  head:  per strip: DVE relu, mul, reduce; partition mixdown via Emat
         matmul; ACT sigmoid; DMA out.
"""

import numpy as np
from contextlib import ExitStack

import concourse.bass as bass
import concourse.bacc as bacc
import concourse.mybir as mybir
import concourse.tile as tile
from concourse.bass_utils import run_bass_kernel_spmd

dt = mybir.dt

U, K, POOL, STRIDE, FC = 300, 19, 7, 7, 100
B, L, D = 256, 1000, 4
P = 140                     # pooled positions per row
EPS = 1e-5
NCORES = 8
BS = B // NCORES            # 32 rows per core
KD = K * D                  # 76 contraction
PA = 127                    # pool windows in the A-chunk (+1 const row = 128)
PB = P - PA                 # 13 windows in the B-chunk
CH = [(0, 300)]              # single conv pass
NG = U // 4                 # FC2 4-unit groups

_COMPILED = None


def _build():
    nc = bacc.Bacc("TRN2", target_bir_lowering=False, debug=False,
                   num_devices=NCORES)

    f16, f32 = dt.float16, dt.float32
    AF = mybir.ActivationFunctionType
    ALU = mybir.AluOpType

    xcol_d = nc.dram_tensor("xcol", [KD, BS, 980], f16, kind="ExternalInput").ap()
    xcolb_d = nc.dram_tensor("xcolb", [KD, 4, 7, 104], f16, kind="ExternalInput").ap()
    wc_d = nc.dram_tensor("wc", [KD, U], f16, kind="ExternalInput").ap()
    w1a_d = nc.dram_tensor("w1a", [128, U, FC], f16, kind="ExternalInput").ap()
    w1b_d = nc.dram_tensor("w1b", [64 + PB, FC, FC], f16, kind="ExternalInput").ap()
    w2_d = nc.dram_tensor("w2s", [FC + 1, NG, 4], f16, kind="ExternalInput").ap()
    wout_d = nc.dram_tensor("woute", [128, NG], f32, kind="ExternalInput").ap()
    E_d = nc.dram_tensor("Emat", [128, BS], f32, kind="ExternalInput").ap()
    ones_d = nc.dram_tensor("onesrow", [1, BS * U], f16, kind="ExternalInput").ap()
    bout_d = nc.dram_tensor("bout", [1, 1], f32, kind="ExternalInput").ap()
    out_d = nc.dram_tensor("out", [1, BS], f32, kind="ExternalOutput").ap()

    with ExitStack() as ctx:
        tc = ctx.enter_context(tile.TileContext(nc))
        consts = ctx.enter_context(tc.tile_pool(name="consts", bufs=1))

        wc = consts.tile([KD, U], f16)
        xcolb = consts.tile([KD, 4, 7, 104], f16)
        w1a = consts.tile([128, U, FC], f16)
        w1b = consts.tile([64 + PB, FC, FC], f16)  # 3 units at partition 0/32/64
        w2s = consts.tile([FC + 1, NG, 4], f16)
        woute = consts.tile([128, NG], f32)
        Emat = consts.tile([128, BS], f32)
        bout = consts.tile([1, 1], f32)
        pexp = consts.tile([128, BS, U], f16)       # [p(127)+const, b, u]
        pexpB = consts.tile([104, 4, U], f16)       # [(rr,pb), blk, u]
        pexpB2 = consts.tile([64 + PB, BS, U], f16)  # [pb, b, u] replicated @0/32/64

        nc.sync.dma_start(wc[:], wc_d[:])
        for bq in range(4):
            nc.sync.dma_start(xcolb[:, bq, :, :], xcolb_d[:, bq, :, :])
        nc.sync.dma_start(pexp[127:128, :, :].rearrange("p b u -> p (b u)"),
                            ones_d[:])

        # ---------------- conv + pool + exp ---------------------------------
        # Only DVE and ACT can read PSUM (Pool engine is DMA-only; a
        # TensorTensor may use at most one PSUM input). Per row: j0-2 land in
        # a 3-bank tile -> DVE reduce_max; j3-6 land in a 4-bank tile -> one
        # batched ACT copy to fp16 SBUF. The 5 partials merge on DVE as a
        # pair-max tree in fp16 2x mode, batched over 4 rows. Merges are
        # emitted one 4-row batch late and exps two batches late so neither
        # ever parks in the 4-deep wait queues and stalls its sequencer.
        pm = consts.tile([128, 36, U], f16)         # merged pool maxes
        with tc.tile_pool(name="xslab", bufs=3) as xpool, \
             tc.tile_pool(name="redps", bufs=1, space="PSUM") as rpsum, \
             tc.tile_pool(name="cpys", bufs=1, space="PSUM") as cpsum, \
             tc.tile_pool(name="qpool", bufs=2) as qpool, \
             tc.tile_pool(name="mpool", bufs=1) as mpool:


            for sq in range(30):
                nc.gpsimd.dma_start(w1a[:, 10 * sq:10 * sq + 10, :],
                                    w1a_d[:, 10 * sq:10 * sq + 10, :])
            nc.gpsimd.dma_start(w1b[:], w1b_d[:])
            nc.gpsimd.dma_start(w2s[:], w2_d[:])
            nc.gpsimd.dma_start(woute[:], wout_d[:])
            nc.gpsimd.dma_start(Emat[:], E_d[:])
            nc.gpsimd.dma_start(bout[:], bout_d[:])

            def conv_pool(lhs7, m, Q, C, qi):
                t3 = rpsum.tile([128, 3, 512], f32, tag="t3")
                for j in range(3):
                    nc.tensor.matmul(t3[0:m, j, 0:U], lhs7[j], wc[:],
                                     start=True, stop=True)
                t4 = cpsum.tile([128, 4, 512], f32, tag="t4")
                for j in range(3, 7):
                    nc.tensor.matmul(t4[0:m, j - 3, 0:U], lhs7[j], wc[:],
                                     start=True, stop=True)
                nc.vector.reduce_max(
                    Q[0:m, qi, :], t3[0:m, :, 0:U].rearrange("p j u -> p u j"),
                    axis=mybir.AxisListType.X)
                nc.scalar.activation(C[0:m, qi, :, :], t4[0:m, 0:4, 0:U], AF.Copy)

            def merges(Q, C, m, s0):
                m2 = mpool.tile([128, 4, 2, 300], f16, tag="m2")
                nc.vector.tensor_max(m2[0:m, :, :, :], C[0:m, :, 0:2, :],
                                     C[0:m, :, 2:4, :])
                m3 = mpool.tile([128, 4, 300], f16, tag="m3")
                nc.vector.tensor_max(m3[0:m, :, :], m2[0:m, :, 0, :],
                                     m2[0:m, :, 1, :])
                nc.vector.tensor_max(pm[0:m, s0:s0 + 4, :], m3[0:m, :, :],
                                     Q[0:m, :, :])

            batches = []     # (Q, C, m, pm-slot, exp destination)
            done_m, done_e = 0, 0

            def drain(upto_m, upto_e):
                nonlocal done_m, done_e
                while done_m < upto_m:
                    Q, C, m, s0, _ = batches[done_m]
                    merges(Q, C, m, s0)
                    done_m += 1
                while done_e < upto_e:
                    _, _, m, s0, dst = batches[done_e]
                    nc.scalar.activation(dst, pm[0:m, s0:s0 + 4, :], AF.Exp)
                    done_e += 1

            # B blocks first so their repack DMAs hide under the A rows
            Q = qpool.tile([128, 4, 300], f16, tag="Q")
            C = qpool.tile([128, 4, 4, 300], f16, tag="C")
            for blk in range(4):
                lhs7 = [xcolb[:, blk, j, :] for j in range(7)]
                conv_pool(lhs7, 8 * PB, Q, C, blk)
            batches.append((Q, C, 8 * PB, 32, pexpB[0:8 * PB, 0:4, :]))
            for sb in range(8):
                slab = xpool.tile([KD, 4, 980], f16, tag="slab")
                nc.sync.dma_start(slab[:], xcol_d[:, 4 * sb:4 * sb + 4, :])
                Q = qpool.tile([128, 4, 300], f16, tag="Q")
                C = qpool.tile([128, 4, 4, 300], f16, tag="C")
                slabr = slab[:].rearrange("q r (p j) -> q r p j", j=7)
                for r in range(4):
                    lhs7 = [slabr[:, r, 0:PA, j] for j in range(7)]
                    conv_pool(lhs7, PA, Q, C, r)
                    if r == 1:
                        drain(sb + 1, sb + 1)
                if sb == 1:
                    # repack pexpB[(rr,pb), blk, u] -> pexpB2[pb, 8*blk+rr, u]
                    for rr in range(8):
                        nc.sync.dma_start(pexpB2[0:PB, rr:BS:8, :],
                                          pexpB[rr * PB:(rr + 1) * PB, :, :])
                    for off in (32, 64):
                        nc.sync.dma_start(pexpB2[off:off + PB, :, :],
                                          pexpB2[0:PB, :, :])
                batches.append((Q, C, PA, 4 * sb,
                                pexp[0:PA, 4 * sb:4 * sb + 4, :]))
            drain(9, 9)

        # ---------------- FC1 + relu + FC2 + head ---------------------------
        fcpool = ctx.enter_context(tc.tile_pool(name="fcsb", bufs=1))
        hrelu = fcpool.tile([FC + 1, U, BS], f16)   # [f+const, u, b]
        nc.sync.dma_start(hrelu[FC:FC + 1, :, :].rearrange("p u b -> p (u b)"),
                            ones_d[:])
        zps_pool = ctx.enter_context(tc.tile_pool(name="zpsp", bufs=1, space="PSUM"))
        zps = zps_pool.tile([128, U], f32)

        with tc.tile_pool(name="fcps", bufs=3, space="PSUM") as fpsum:
            for ci, (c0, w) in enumerate(CH):
                for g in range((w + 15) // 16):
                    u0 = c0 + 16 * g
                    nun = min(16, c0 + w - u0)
                    hps = fpsum.tile([FC, 16, BS], f32, tag="hps")
                    for s in range(nun):
                        u = u0 + s
                        o = hps[0:FC, s, 0:BS]
                        nc.tensor.matmul(o, w1a[:, u, :], pexp[:, :, u],
                                         start=True, stop=False)
                        off = 32 * (u % 3)
                        nc.tensor.matmul(
                            o, w1b[off:off + PB, u // 3, :],
                            pexpB2[off:off + PB, :, u], start=False, stop=True)
                    if g % 2:
                        nc.vector.tensor_scalar_max(hrelu[0:FC, u0:u0 + nun, :],
                                                    hps[0:FC, 0:nun, 0:BS], 0.0)
                    else:
                        nc.scalar.activation(hrelu[0:FC, u0:u0 + nun, :],
                                             hps[0:FC, 0:nun, 0:BS], AF.Relu)
                    for k in range(u0 // 4, (u0 + nun) // 4):
                        nc.tensor.matmul(
                            zps[0:128, 4 * k:4 * k + 4],
                            hrelu[0:FC + 1, 4 * k:4 * k + 4, :].rearrange(
                                "f u b -> f (u b)"),
                            w2s[:, k, :], start=True, stop=True)

            # head: per strip n: relu then fused mul+reduce over u
            part = fcpool.tile([128, 1], f32)
            zr = fcpool.tile([128, NG], f32)
            prod = fcpool.tile([128, NG], f32)
            for n in range(4):
                sl = slice(32 * n, 32 * n + 32)
                nc.vector.tensor_scalar_max(zr[sl, :], zps[sl, n:U:4], 0.0)
                nc.vector.tensor_mul(prod[sl, :], zr[sl, :], woute[sl, :])
                nc.vector.tensor_reduce(part[sl, 0:1], prod[sl, :],
                                        axis=mybir.AxisListType.X, op=ALU.add)
            with tc.tile_pool(name="headps", bufs=1, space="PSUM") as hpsum:
                zf = hpsum.tile([1, BS], f32, tag="zf")
                nc.tensor.matmul(zf[0:1, :], part[:], Emat[:], start=True, stop=True)
                osb = fcpool.tile([1, BS], f32)
                nc.scalar.activation(osb[:], zf[0:1, :], AF.Sigmoid, bias=bout[0:1, :])
                nc.sync.dma_start(out_d[:], osb[:])

    nc.compile()
    return nc


def _prep_weights(i):
    """Host-side BN folding + layout. numpy fp32 math -> fp16 payloads."""
    f = lambda a: np.asarray(a, np.float32)
    w_conv, b_conv = f(i["w_conv"]), f(i["b_conv"])
    g1, be1, m1, v1 = f(i["g1"]), f(i["be1"]), f(i["m1"]), f(i["v1"])
    w_fc1, b_fc1 = f(i["w_fc1"]), f(i["b_fc1"])
    g2, be2, m2, v2 = f(i["g2"]), f(i["be2"]), f(i["m2"]), f(i["v2"])
    w_fc2, b_fc2 = f(i["w_fc2"]), f(i["b_fc2"])
    g3, be3, m3, v3 = f(i["g3"]), f(i["be3"]), f(i["m3"]), f(i["v3"])
    w_out, b_out = f(i["w_out"]), f(i["b_out"])

    s1 = g1 / np.sqrt(v1 + EPS)
    t1 = be1 - m1 * s1
    s2 = g2 / np.sqrt(v2 + EPS)
    b1pp = (b_fc1 - m2) * s2 + be2
    s3 = g3 / np.sqrt(v3 + EPS)
    w2pp = w_fc2 * s3[:, None]
    b2pp = (b_fc2 - m3) * s3 + be3

    # conv weights with BN1 scale folded; contraction index q = k*D + d
    Wc = np.ascontiguousarray(
        (w_conv * s1[:, None, None]).transpose(2, 1, 0).reshape(KD, U))
    # FC1 with BN2 scale and exp(t1 + s1*b_conv) folded
    gexp = np.exp(t1 + s1 * b_conv)
    w1pp = (w_fc1 * s2[:, :, None] * gexp[:, None, None]).transpose(2, 0, 1)  # (P,U,FC)
    w1a = np.empty((128, U, FC), np.float32)
    w1a[:PA] = w1pp[:PA]
    w1a[127] = b1pp                      # bias rides the const-1 pexp row
    # w1b: 3 units per partition group at offsets 0/32/64: [32*(u%3)+pb, u//3, f]
    w1b = np.zeros((64 + PB, FC, FC), np.float32)
    for u in range(U):
        w1b[32 * (u % 3):32 * (u % 3) + PB, u // 3] = w1pp[PA:P, u]

    # FC2 weights f-major with bias row: w2s[f, k, n] = w2pp[4k+n, f]
    w2s = np.empty((FC + 1, NG, 4), np.float32)
    w2s[:FC] = w2pp.T.reshape(FC, NG, 4)
    w2s[FC] = b2pp.reshape(NG, 4)

    # head: strip n rows 32n..32n+32 hold w_out[n::4]
    woute = np.zeros((128, NG), np.float32)
    for n in range(4):
        woute[32 * n:32 * n + 32] = w_out[n::4, 0][None]
    Em = np.zeros((128, BS), np.float32)
    for n in range(4):
        Em[32 * n:32 * n + 32] = np.eye(BS, dtype=np.float32)

    h16 = lambda a: np.asarray(a, np.float16)
    return {
        "wc": h16(Wc), "w1a": h16(w1a), "w1b": h16(w1b), "w2s": h16(w2s),
        "woute": woute, "Emat": Em,
        "onesrow": np.ones((1, BS * U), np.float16),
        "bout": np.asarray(b_out, np.float32).reshape(1, 1),
    }


def kernel(**inputs) -> np.ndarray:
    global _COMPILED
    if _COMPILED is None:
        _COMPILED = _build()
    nc = _COMPILED

    wmap = _prep_weights(inputs)
    x = np.asarray(inputs["input_seq"], np.float32)   # (256, 1000, 4)
    win = np.lib.stride_tricks.sliding_window_view(x, K, axis=1)  # (B, 982, D, K)
    in_maps = []
    for c in range(NCORES):
        xs = win[c * BS:(c + 1) * BS, :980]           # (32, 980, 4, 19)
        xcol = np.ascontiguousarray(
            xs.transpose(3, 2, 0, 1).astype(np.float16)).reshape(KD, BS, 980)
        tail = xcol[:, :, 7 * PA:].reshape(KD, 4, 8, PB, 7)
        xcolb = np.ascontiguousarray(tail.transpose(0, 1, 4, 2, 3)).reshape(KD, 4, 7, 104)
        in_maps.append({"xcol": xcol, "xcolb": xcolb, **wmap})

    res = run_bass_kernel_spmd(nc, in_maps, list(range(NCORES)))
    out = np.empty((B, 1), np.float32)
    for c in range(NCORES):
        out[c * BS:(c + 1) * BS, 0] = res.results[c]["out"][0]
    return out


# revision 16
# speedup vs baseline: 1.0628x; 1.0071x over previous
"""ExplaiNN Trainium2 kernel — 8-core SPMD, batch-sharded (32 rows/core).

Restructured from the 154.5us baseline (cost-model findings: DVE 106us busy
on fp32 reduce-max pooling + FC2; Pool engine 65us of SWDGE descriptor
generation; PE 41% busy). Now 113.4us with rel err 3.1e-3 (was 1.5e-2).

  dtype: fp16 matmul operands everywhere (vs fp32r/bf16) — halves DMA and
         keeps full-rate PE matmuls at any stream width, with better
         precision than bf16.
  conv:  X-stationary matmuls as before; per row j0-2 land in a 3-bank PSUM
         tile, j3-6 in a 4-bank tile. B-blocks (tail 13 pool windows x 8
         rows) run FIRST so their pexpB2 repack DMA chain hides under the
         A-row stream.
  pool:  only DVE and ACT can read PSUM (the Pool engine is DMA-only and a
         TensorTensor allows at most one PSUM input — hw limits). DVE
         reduce_max eats j0-2; one batched ACT copy moves j3-6 to fp16
         SBUF; DVE merges the 5 partials as a pair-max tree in fp16 2x
         mode, batched over 4 rows; batched ACT exp writes pexp. Merges are
         emitted one 4-row batch late and exps a batch later so they never
         park in the 4-deep wait queues and stall their sequencer (convoy
         avoidance); weight prefetch rides the otherwise-idle Pool queue.
  FC1:   weight-stationary per unit (ldweights w1 [128,100], stream pexp
         [128,32]) -> h lands f-major [100f, 32b] in PSUM, 16 units/bank,
         triple-buffered; relu alternates DVE/ACT -> hrelu fp16 (+const-1 row 100 carrying
         the FC2 bias). K-tail weights w1b packed 3 units per partition
         group at offsets 0/32/64 (matmul base-partition constraint), with
         pexpB2 replicated to those offsets.
  FC2:   per 4-unit group one PE pair: lhsT = hrelu [101, 4ux32b], rhs =
         w2 [101, 4] -> block-diagonal strips of zps [128, 300].
  head:  per strip: DVE relu, mul, reduce; partition mixdown via Emat
         matmul; ACT sigmoid; DMA out.
"""

import numpy as np
from contextlib import ExitStack

import concourse.bass as bass
import concourse.bacc as bacc
import concourse.mybir as mybir
import concourse.tile as tile
from concourse.bass_utils import run_bass_kernel_spmd

dt = mybir.dt

U, K, POOL, STRIDE, FC = 300, 19, 7, 7, 100
B, L, D = 256, 1000, 4
P = 140                     # pooled positions per row
EPS = 1e-5
NCORES = 8
BS = B // NCORES            # 32 rows per core
KD = K * D                  # 76 contraction
PA = 127                    # pool windows in the A-chunk (+1 const row = 128)
PB = P - PA                 # 13 windows in the B-chunk
CH = [(0, 300)]              # single conv pass
NG = U // 4                 # FC2 4-unit groups

_COMPILED = None


def _build():
    nc = bacc.Bacc("TRN2", target_bir_lowering=False, debug=False,
                   num_devices=NCORES)

    f16, f32 = dt.float16, dt.float32
    AF = mybir.ActivationFunctionType
    ALU = mybir.AluOpType

    xcol_d = nc.dram_tensor("xcol", [KD, BS, 980], f16, kind="ExternalInput").ap()
    xcolb_d = nc.dram_tensor("xcolb", [KD, 4, 7, 104], f16, kind="ExternalInput").ap()
    wc_d = nc.dram_tensor("wc", [KD, U], f16, kind="ExternalInput").ap()
    w1a_d = nc.dram_tensor("w1a", [128, U, FC], f16, kind="ExternalInput").ap()
    w1b_d = nc.dram_tensor("w1b", [64 + PB, FC, FC], f16, kind="ExternalInput").ap()
    w2_d = nc.dram_tensor("w2s", [FC + 1, NG, 4], f16, kind="ExternalInput").ap()
    wout_d = nc.dram_tensor("woute", [128, U], f32, kind="ExternalInput").ap()
    E_d = nc.dram_tensor("Emat", [128, BS], f32, kind="ExternalInput").ap()
    ones_d = nc.dram_tensor("onesrow", [1, BS * U], f16, kind="ExternalInput").ap()
    bout_d = nc.dram_tensor("bout", [1, 1], f32, kind="ExternalInput").ap()
    out_d = nc.dram_tensor("out", [1, BS], f32, kind="ExternalOutput").ap()

    with ExitStack() as ctx:
        tc = ctx.enter_context(tile.TileContext(nc))
        consts = ctx.enter_context(tc.tile_pool(name="consts", bufs=1))

        wc = consts.tile([KD, U], f16)
        xcolb = consts.tile([KD, 4, 7, 104], f16)
        w1a = consts.tile([128, U, FC], f16)
        w1b = consts.tile([64 + PB, FC, FC], f16)  # 3 units at partition 0/32/64
        w2s = consts.tile([FC + 1, NG, 4], f16)
        woute = consts.tile([128, U], f32)
        Emat = consts.tile([128, BS], f32)
        bout = consts.tile([1, 1], f32)
        pexp = consts.tile([128, BS, U], f16)       # [p(127)+const, b, u]
        pexpB = consts.tile([104, 4, U], f16)       # [(rr,pb), blk, u]
        pexpB2 = consts.tile([64 + PB, BS, U], f16)  # [pb, b, u] replicated @0/32/64

        nc.sync.dma_start(wc[:], wc_d[:])
        for bq in range(4):
            nc.sync.dma_start(xcolb[:, bq, :, :], xcolb_d[:, bq, :, :])
        nc.sync.dma_start(pexp[127:128, :, :].rearrange("p b u -> p (b u)"),
                            ones_d[:])

        # ---------------- conv + pool + exp ---------------------------------
        # Only DVE and ACT can read PSUM (Pool engine is DMA-only; a
        # TensorTensor may use at most one PSUM input). Per row: j0-2 land in
        # a 3-bank tile -> DVE reduce_max; j3-6 land in a 4-bank tile -> one
        # batched ACT copy to fp16 SBUF. The 5 partials merge on DVE as a
        # pair-max tree in fp16 2x mode, batched over 4 rows. Merges are
        # emitted one 4-row batch late and exps two batches late so neither
        # ever parks in the 4-deep wait queues and stalls its sequencer.
        pm = consts.tile([128, 36, U], f16)         # merged pool maxes
        with tc.tile_pool(name="xslab", bufs=3) as xpool, \
             tc.tile_pool(name="redps", bufs=1, space="PSUM") as rpsum, \
             tc.tile_pool(name="cpys", bufs=1, space="PSUM") as cpsum, \
             tc.tile_pool(name="qpool", bufs=2) as qpool, \
             tc.tile_pool(name="mpool", bufs=1) as mpool:


            for sq in range(30):
                nc.gpsimd.dma_start(w1a[:, 10 * sq:10 * sq + 10, :],
                                    w1a_d[:, 10 * sq:10 * sq + 10, :])
            nc.gpsimd.dma_start(w1b[:], w1b_d[:])
            nc.gpsimd.dma_start(w2s[:], w2_d[:])
            nc.gpsimd.dma_start(woute[:], wout_d[:])
            nc.gpsimd.dma_start(Emat[:], E_d[:])
            nc.gpsimd.dma_start(bout[:], bout_d[:])

            def conv_pool(lhs7, m, Q, C, qi):
                t3 = rpsum.tile([128, 3, 512], f32, tag="t3")
                for j in range(3):
                    nc.tensor.matmul(t3[0:m, j, 0:U], lhs7[j], wc[:],
                                     start=True, stop=True)
                t4 = cpsum.tile([128, 4, 512], f32, tag="t4")
                for j in range(3, 7):
                    nc.tensor.matmul(t4[0:m, j - 3, 0:U], lhs7[j], wc[:],
                                     start=True, stop=True)
                nc.vector.reduce_max(
                    Q[0:m, qi, :], t3[0:m, :, 0:U].rearrange("p j u -> p u j"),
                    axis=mybir.AxisListType.X)
                nc.scalar.activation(C[0:m, qi, :, :], t4[0:m, 0:4, 0:U], AF.Copy)

            def merges(Q, C, m, s0):
                m2 = mpool.tile([128, 4, 2, 300], f16, tag="m2")
                nc.vector.tensor_max(m2[0:m, :, :, :], C[0:m, :, 0:2, :],
                                     C[0:m, :, 2:4, :])
                m3 = mpool.tile([128, 4, 300], f16, tag="m3")
                nc.vector.tensor_max(m3[0:m, :, :], m2[0:m, :, 0, :],
                                     m2[0:m, :, 1, :])
                nc.vector.tensor_max(pm[0:m, s0:s0 + 4, :], m3[0:m, :, :],
                                     Q[0:m, :, :])

            batches = []     # (Q, C, m, pm-slot, exp destination)
            done_m, done_e = 0, 0

            def drain(upto_m, upto_e):
                nonlocal done_m, done_e
                while done_m < upto_m:
                    Q, C, m, s0, _ = batches[done_m]
                    merges(Q, C, m, s0)
                    done_m += 1
                while done_e < upto_e:
                    _, _, m, s0, dst = batches[done_e]
                    nc.scalar.activation(dst, pm[0:m, s0:s0 + 4, :], AF.Exp)
                    done_e += 1

            # B blocks first so their repack DMAs hide under the A rows
            Q = qpool.tile([128, 4, 300], f16, tag="Q")
            C = qpool.tile([128, 4, 4, 300], f16, tag="C")
            for blk in range(4):
                lhs7 = [xcolb[:, blk, j, :] for j in range(7)]
                conv_pool(lhs7, 8 * PB, Q, C, blk)
            batches.append((Q, C, 8 * PB, 32, pexpB[0:8 * PB, 0:4, :]))
            for sb in range(8):
                slab = xpool.tile([KD, 4, 980], f16, tag="slab")
                nc.sync.dma_start(slab[:], xcol_d[:, 4 * sb:4 * sb + 4, :])
                Q = qpool.tile([128, 4, 300], f16, tag="Q")
                C = qpool.tile([128, 4, 4, 300], f16, tag="C")
                slabr = slab[:].rearrange("q r (p j) -> q r p j", j=7)
                for r in range(4):
                    lhs7 = [slabr[:, r, 0:PA, j] for j in range(7)]
                    conv_pool(lhs7, PA, Q, C, r)
                    if r == 1:
                        drain(sb + 1, sb + 1)
                if sb == 1:
                    # repack pexpB[(rr,pb), blk, u] -> pexpB2[pb, 8*blk+rr, u]
                    for rr in range(8):
                        nc.sync.dma_start(pexpB2[0:PB, rr:BS:8, :],
                                          pexpB[rr * PB:(rr + 1) * PB, :, :])
                    for off in (32, 64):
                        nc.sync.dma_start(pexpB2[off:off + PB, :, :],
                                          pexpB2[0:PB, :, :])
                batches.append((Q, C, PA, 4 * sb,
                                pexp[0:PA, 4 * sb:4 * sb + 4, :]))
            drain(9, 9)

        # ---------------- FC1 + relu + FC2 + head ---------------------------
        fcpool = ctx.enter_context(tc.tile_pool(name="fcsb", bufs=1))
        hrelu = fcpool.tile([FC + 1, U, BS], f16)   # [f+const, u, b]
        nc.sync.dma_start(hrelu[FC:FC + 1, :, :].rearrange("p u b -> p (u b)"),
                            ones_d[:])
        zps_pool = ctx.enter_context(tc.tile_pool(name="zpsp", bufs=1, space="PSUM"))
        zps = zps_pool.tile([128, U], f32)

        with tc.tile_pool(name="fcps", bufs=3, space="PSUM") as fpsum:
            for ci, (c0, w) in enumerate(CH):
                for g in range((w + 15) // 16):
                    u0 = c0 + 16 * g
                    nun = min(16, c0 + w - u0)
                    hps = fpsum.tile([FC, 16, BS], f32, tag="hps")
                    for s in range(nun):
                        u = u0 + s
                        o = hps[0:FC, s, 0:BS]
                        nc.tensor.matmul(o, w1a[:, u, :], pexp[:, :, u],
                                         start=True, stop=False)
                        off = 32 * (u % 3)
                        nc.tensor.matmul(
                            o, w1b[off:off + PB, u // 3, :],
                            pexpB2[off:off + PB, :, u], start=False, stop=True)
                    if g % 2:
                        nc.vector.tensor_scalar_max(hrelu[0:FC, u0:u0 + nun, :],
                                                    hps[0:FC, 0:nun, 0:BS], 0.0)
                    else:
                        nc.scalar.activation(hrelu[0:FC, u0:u0 + nun, :],
                                             hps[0:FC, 0:nun, 0:BS], AF.Relu)
                    for k in range(u0 // 4, (u0 + nun) // 4):
                        nc.tensor.matmul(
                            zps[0:128, 4 * k:4 * k + 4],
                            hrelu[0:FC + 1, 4 * k:4 * k + 4, :].rearrange(
                                "f u b -> f (u b)"),
                            w2s[:, k, :], start=True, stop=True)

            # head: full-width relu/mul/reduce; woute zero-masks the
            # off-diagonal strip lanes so they vanish in the reduce
            part = fcpool.tile([128, 1], f32)
            zr = fcpool.tile([128, U], f32)
            prod = fcpool.tile([128, U], f32)
            nc.vector.tensor_scalar_max(zr[:], zps[:], 0.0)
            nc.vector.tensor_mul(prod[:], zr[:], woute[:])
            nc.vector.tensor_reduce(part[:, 0:1], prod[:],
                                    axis=mybir.AxisListType.X, op=ALU.add)
            with tc.tile_pool(name="headps", bufs=1, space="PSUM") as hpsum:
                zf = hpsum.tile([1, BS], f32, tag="zf")
                nc.tensor.matmul(zf[0:1, :], part[:], Emat[:], start=True, stop=True)
                osb = fcpool.tile([1, BS], f32)
                nc.scalar.activation(osb[:], zf[0:1, :], AF.Sigmoid, bias=bout[0:1, :])
                nc.sync.dma_start(out_d[:], osb[:])

    nc.compile()
    return nc


def _prep_weights(i):
    """Host-side BN folding + layout. numpy fp32 math -> fp16 payloads."""
    f = lambda a: np.asarray(a, np.float32)
    w_conv, b_conv = f(i["w_conv"]), f(i["b_conv"])
    g1, be1, m1, v1 = f(i["g1"]), f(i["be1"]), f(i["m1"]), f(i["v1"])
    w_fc1, b_fc1 = f(i["w_fc1"]), f(i["b_fc1"])
    g2, be2, m2, v2 = f(i["g2"]), f(i["be2"]), f(i["m2"]), f(i["v2"])
    w_fc2, b_fc2 = f(i["w_fc2"]), f(i["b_fc2"])
    g3, be3, m3, v3 = f(i["g3"]), f(i["be3"]), f(i["m3"]), f(i["v3"])
    w_out, b_out = f(i["w_out"]), f(i["b_out"])

    s1 = g1 / np.sqrt(v1 + EPS)
    t1 = be1 - m1 * s1
    s2 = g2 / np.sqrt(v2 + EPS)
    b1pp = (b_fc1 - m2) * s2 + be2
    s3 = g3 / np.sqrt(v3 + EPS)
    w2pp = w_fc2 * s3[:, None]
    b2pp = (b_fc2 - m3) * s3 + be3

    # conv weights with BN1 scale folded; contraction index q = k*D + d
    Wc = np.ascontiguousarray(
        (w_conv * s1[:, None, None]).transpose(2, 1, 0).reshape(KD, U))
    # FC1 with BN2 scale and exp(t1 + s1*b_conv) folded
    gexp = np.exp(t1 + s1 * b_conv)
    w1pp = (w_fc1 * s2[:, :, None] * gexp[:, None, None]).transpose(2, 0, 1)  # (P,U,FC)
    w1a = np.empty((128, U, FC), np.float32)
    w1a[:PA] = w1pp[:PA]
    w1a[127] = b1pp                      # bias rides the const-1 pexp row
    # w1b: 3 units per partition group at offsets 0/32/64: [32*(u%3)+pb, u//3, f]
    w1b = np.zeros((64 + PB, FC, FC), np.float32)
    for u in range(U):
        w1b[32 * (u % 3):32 * (u % 3) + PB, u // 3] = w1pp[PA:P, u]

    # FC2 weights f-major with bias row: w2s[f, k, n] = w2pp[4k+n, f]
    w2s = np.empty((FC + 1, NG, 4), np.float32)
    w2s[:FC] = w2pp.T.reshape(FC, NG, 4)
    w2s[FC] = b2pp.reshape(NG, 4)

    # head: full-width, zero-masked: strip n rows keep only cols n::4
    woute = np.zeros((128, U), np.float32)
    for n in range(4):
        woute[32 * n:32 * n + 32, n::4] = w_out[n::4, 0][None]
    Em = np.zeros((128, BS), np.float32)
    for n in range(4):
        Em[32 * n:32 * n + 32] = np.eye(BS, dtype=np.float32)

    h16 = lambda a: np.asarray(a, np.float16)
    return {
        "wc": h16(Wc), "w1a": h16(w1a), "w1b": h16(w1b), "w2s": h16(w2s),
        "woute": woute, "Emat": Em,
        "onesrow": np.ones((1, BS * U), np.float16),
        "bout": np.asarray(b_out, np.float32).reshape(1, 1),
    }


def kernel(**inputs) -> np.ndarray:
    global _COMPILED
    if _COMPILED is None:
        _COMPILED = _build()
    nc = _COMPILED

    wmap = _prep_weights(inputs)
    x = np.asarray(inputs["input_seq"], np.float32)   # (256, 1000, 4)
    win = np.lib.stride_tricks.sliding_window_view(x, K, axis=1)  # (B, 982, D, K)
    in_maps = []
    for c in range(NCORES):
        xs = win[c * BS:(c + 1) * BS, :980]           # (32, 980, 4, 19)
        xcol = np.ascontiguousarray(
            xs.transpose(3, 2, 0, 1).astype(np.float16)).reshape(KD, BS, 980)
        tail = xcol[:, :, 7 * PA:].reshape(KD, 4, 8, PB, 7)
        xcolb = np.ascontiguousarray(tail.transpose(0, 1, 4, 2, 3)).reshape(KD, 4, 7, 104)
        in_maps.append({"xcol": xcol, "xcolb": xcolb, **wmap})

    res = run_bass_kernel_spmd(nc, in_maps, list(range(NCORES)))
    out = np.empty((B, 1), np.float32)
    for c in range(NCORES):
        out[c * BS:(c + 1) * BS, 0] = res.results[c]["out"][0]
    return out


# revision 18
# speedup vs baseline: 1.0726x; 1.0092x over previous
"""ExplaiNN Trainium2 kernel — 8-core SPMD, batch-sharded (32 rows/core).

Restructured from the 154.5us baseline (cost-model findings: DVE 106us busy
on fp32 reduce-max pooling + FC2; Pool engine 65us of SWDGE descriptor
generation; PE 41% busy). Now 113.4us with rel err 3.1e-3 (was 1.5e-2).

  dtype: fp16 matmul operands everywhere (vs fp32r/bf16) — halves DMA and
         keeps full-rate PE matmuls at any stream width, with better
         precision than bf16.
  conv:  X-stationary matmuls as before; per row j0-2 land in a 3-bank PSUM
         tile, j3-6 in a 4-bank tile. B-blocks (tail 13 pool windows x 8
         rows) run FIRST so their pexpB2 repack DMA chain hides under the
         A-row stream.
  pool:  only DVE and ACT can read PSUM (the Pool engine is DMA-only and a
         TensorTensor allows at most one PSUM input — hw limits). DVE
         reduce_max eats j0-2; one batched ACT copy moves j3-6 to fp16
         SBUF; DVE merges the 5 partials as a pair-max tree in fp16 2x
         mode, batched over 4 rows; batched ACT exp writes pexp. Merges are
         emitted one 4-row batch late and exps a batch later so they never
         park in the 4-deep wait queues and stall their sequencer (convoy
         avoidance); weight prefetch rides the otherwise-idle Pool queue.
  FC1:   weight-stationary per unit (ldweights w1 [128,100], stream pexp
         [128,32]) -> h lands f-major [100f, 32b] in PSUM, 16 units/bank,
         triple-buffered; relu alternates DVE/ACT -> hrelu fp16 (+const-1 row 100 carrying
         the FC2 bias). K-tail weights w1b packed 3 units per partition
         group at offsets 0/32/64 (matmul base-partition constraint), with
         pexpB2 replicated to those offsets.
  FC2:   per 4-unit group one PE pair: lhsT = hrelu [101, 4ux32b], rhs =
         w2 [101, 4] -> block-diagonal strips of zps [128, 300].
  head:  per strip: DVE relu, mul, reduce; partition mixdown via Emat
         matmul; ACT sigmoid; DMA out.
"""

import numpy as np
from contextlib import ExitStack

import concourse.bass as bass
import concourse.bacc as bacc
import concourse.mybir as mybir
import concourse.tile as tile
from concourse.bass_utils import run_bass_kernel_spmd

dt = mybir.dt

U, K, POOL, STRIDE, FC = 300, 19, 7, 7, 100
B, L, D = 256, 1000, 4
P = 140                     # pooled positions per row
EPS = 1e-5
NCORES = 8
BS = B // NCORES            # 32 rows per core
KD = K * D                  # 76 contraction
PA = 127                    # pool windows in the A-chunk (+1 const row = 128)
PB = P - PA                 # 13 windows in the B-chunk
CH = [(0, 300)]              # single conv pass
NG = U // 4                 # FC2 4-unit groups

_COMPILED = None


def _build():
    nc = bacc.Bacc("TRN2", target_bir_lowering=False, debug=False,
                   num_devices=NCORES)

    f16, f32 = dt.float16, dt.float32
    AF = mybir.ActivationFunctionType
    ALU = mybir.AluOpType

    xcol_d = nc.dram_tensor("xcol", [KD, BS, 980], f16, kind="ExternalInput").ap()
    xcolb_d = nc.dram_tensor("xcolb", [KD, 4, 7, 104], f16, kind="ExternalInput").ap()
    wc_d = nc.dram_tensor("wc", [KD, U], f16, kind="ExternalInput").ap()
    w1a_d = nc.dram_tensor("w1a", [128, U, FC], f16, kind="ExternalInput").ap()
    w1b_d = nc.dram_tensor("w1b", [64 + PB, FC, FC], f16, kind="ExternalInput").ap()
    w2_d = nc.dram_tensor("w2s", [FC + 1, NG, 4], f16, kind="ExternalInput").ap()
    wout_d = nc.dram_tensor("woute", [128, U], f32, kind="ExternalInput").ap()
    E_d = nc.dram_tensor("Emat", [128, BS], f32, kind="ExternalInput").ap()
    ones_d = nc.dram_tensor("onesrow", [1, BS * U], f16, kind="ExternalInput").ap()
    bout_d = nc.dram_tensor("bout", [1, 1], f32, kind="ExternalInput").ap()
    out_d = nc.dram_tensor("out", [1, BS], f32, kind="ExternalOutput").ap()

    with ExitStack() as ctx:
        tc = ctx.enter_context(tile.TileContext(nc))
        consts = ctx.enter_context(tc.tile_pool(name="consts", bufs=1))

        wc = consts.tile([KD, U], f16)
        xcolb = consts.tile([KD, 4, 7, 104], f16)
        w1a = consts.tile([128, U, FC], f16)
        w1b = consts.tile([64 + PB, FC, FC], f16)  # 3 units at partition 0/32/64
        w2s = consts.tile([FC + 1, NG, 4], f16)
        woute = consts.tile([128, U], f32)
        Emat = consts.tile([128, BS], f32)
        bout = consts.tile([1, 1], f32)
        pexp = consts.tile([128, BS, U], f16)       # [p(127)+const, b, u]
        pexpB = consts.tile([104, 4, U], f16)       # [(rr,pb), blk, u]
        pexpB2 = consts.tile([64 + PB, BS, U], f16)  # [pb, b, u] replicated @0/32/64

        nc.sync.dma_start(wc[:], wc_d[:])
        for bq in range(4):
            nc.sync.dma_start(xcolb[:, bq, :, :], xcolb_d[:, bq, :, :])
        nc.sync.dma_start(pexp[127:128, :, :].rearrange("p b u -> p (b u)"),
                            ones_d[:])

        # ---------------- conv + pool + exp ---------------------------------
        # Only DVE and ACT can read PSUM (Pool engine is DMA-only; a
        # TensorTensor may use at most one PSUM input). Per row: j0-2 land in
        # a 3-bank tile -> DVE reduce_max; j3-6 land in a 4-bank tile -> one
        # batched ACT copy to fp16 SBUF. The 5 partials merge on DVE as a
        # pair-max tree in fp16 2x mode, batched over 4 rows. Merges are
        # emitted one 4-row batch late and exps two batches late so neither
        # ever parks in the 4-deep wait queues and stalls its sequencer.
        pm = consts.tile([128, 36, U], f16)         # merged pool maxes
        with tc.tile_pool(name="xslab", bufs=3) as xpool, \
             tc.tile_pool(name="redps", bufs=1, space="PSUM") as rpsum, \
             tc.tile_pool(name="cpys", bufs=1, space="PSUM") as cpsum, \
             tc.tile_pool(name="qpool", bufs=2) as qpool, \
             tc.tile_pool(name="mpool", bufs=1) as mpool:


            for sq in range(30):
                nc.gpsimd.dma_start(w1a[:, 10 * sq:10 * sq + 10, :],
                                    w1a_d[:, 10 * sq:10 * sq + 10, :])
            nc.gpsimd.dma_start(w1b[:], w1b_d[:])
            nc.gpsimd.dma_start(w2s[:], w2_d[:])
            nc.gpsimd.dma_start(woute[:], wout_d[:])
            nc.gpsimd.dma_start(Emat[:], E_d[:])
            nc.gpsimd.dma_start(bout[:], bout_d[:])

            def conv_pool(lhs7, m, Q, C, qi):
                t3 = rpsum.tile([128, 3, 512], f32, tag="t3")
                for j in range(3):
                    nc.tensor.matmul(t3[0:m, j, 0:U], lhs7[j], wc[:],
                                     start=True, stop=True)
                t4 = cpsum.tile([128, 4, 512], f32, tag="t4")
                for j in range(3, 7):
                    nc.tensor.matmul(t4[0:m, j - 3, 0:U], lhs7[j], wc[:],
                                     start=True, stop=True)
                nc.vector.reduce_max(
                    Q[0:m, qi, :], t3[0:m, :, 0:U].rearrange("p j u -> p u j"),
                    axis=mybir.AxisListType.X)
                nc.scalar.activation(C[0:m, qi, :, :], t4[0:m, 0:4, 0:U], AF.Copy)

            def merges(Q, C, m, s0):
                m2 = mpool.tile([128, 4, 2, 300], f16, tag="m2")
                nc.vector.tensor_max(m2[0:m, :, :, :], C[0:m, :, 0:2, :],
                                     C[0:m, :, 2:4, :])
                m3 = mpool.tile([128, 4, 300], f16, tag="m3")
                nc.vector.tensor_max(m3[0:m, :, :], m2[0:m, :, 0, :],
                                     m2[0:m, :, 1, :])
                nc.vector.tensor_max(pm[0:m, s0:s0 + 4, :], m3[0:m, :, :],
                                     Q[0:m, :, :])

            batches = []     # (Q, C, m, pm-slot, exp destination)
            done_m, done_e = 0, 0

            def drain(upto_m, upto_e):
                nonlocal done_m, done_e
                while done_m < upto_m:
                    Q, C, m, s0, _ = batches[done_m]
                    merges(Q, C, m, s0)
                    done_m += 1
                while done_e < upto_e:
                    _, _, m, s0, dst = batches[done_e]
                    nc.scalar.activation(dst[:, :, 0:150], pm[0:m, s0:s0 + 4, 0:150], AF.Exp)
                    nc.scalar.activation(dst[:, :, 150:U], pm[0:m, s0:s0 + 4, 150:U], AF.Exp)
                    done_e += 1

            # B blocks first so their repack DMAs hide under the A rows
            Q = qpool.tile([128, 4, 300], f16, tag="Q")
            C = qpool.tile([128, 4, 4, 300], f16, tag="C")
            for blk in range(4):
                lhs7 = [xcolb[:, blk, j, :] for j in range(7)]
                conv_pool(lhs7, 8 * PB, Q, C, blk)
            batches.append((Q, C, 8 * PB, 32, pexpB[0:8 * PB, 0:4, :]))
            for sb in range(8):
                slab = xpool.tile([KD, 4, 980], f16, tag="slab")
                nc.sync.dma_start(slab[:], xcol_d[:, 4 * sb:4 * sb + 4, :])
                Q = qpool.tile([128, 4, 300], f16, tag="Q")
                C = qpool.tile([128, 4, 4, 300], f16, tag="C")
                slabr = slab[:].rearrange("q r (p j) -> q r p j", j=7)
                for r in range(4):
                    lhs7 = [slabr[:, r, 0:PA, j] for j in range(7)]
                    conv_pool(lhs7, PA, Q, C, r)
                    if r == 3:
                        drain(sb + 1, sb + 1)
                if sb == 1:
                    # repack pexpB[(rr,pb), blk, u] -> pexpB2[pb, 8*blk+rr, u]
                    for rr in range(8):
                        nc.sync.dma_start(pexpB2[0:PB, rr:BS:8, :],
                                          pexpB[rr * PB:(rr + 1) * PB, :, :])
                    for off in (32, 64):
                        nc.sync.dma_start(pexpB2[off:off + PB, :, :],
                                          pexpB2[0:PB, :, :])
                batches.append((Q, C, PA, 4 * sb,
                                pexp[0:PA, 4 * sb:4 * sb + 4, :]))
            drain(9, 9)

        # ---------------- FC1 + relu + FC2 + head ---------------------------
        fcpool = ctx.enter_context(tc.tile_pool(name="fcsb", bufs=1))
        hrelu = fcpool.tile([FC + 1, U, BS], f16)   # [f+const, u, b]
        nc.sync.dma_start(hrelu[FC:FC + 1, :, :].rearrange("p u b -> p (u b)"),
                            ones_d[:])
        zps_pool = ctx.enter_context(tc.tile_pool(name="zpsp", bufs=1, space="PSUM"))
        zps = zps_pool.tile([128, U], f32)

        with tc.tile_pool(name="fcps", bufs=3, space="PSUM") as fpsum:
            for ci, (c0, w) in enumerate(CH):
                for g in range((w + 15) // 16):
                    u0 = c0 + 16 * g
                    nun = min(16, c0 + w - u0)
                    hps = fpsum.tile([FC, 16, BS], f32, tag="hps")
                    for s in range(nun):
                        u = u0 + s
                        o = hps[0:FC, s, 0:BS]
                        nc.tensor.matmul(o, w1a[:, u, :], pexp[:, :, u],
                                         start=True, stop=False)
                        off = 32 * (u % 3)
                        nc.tensor.matmul(
                            o, w1b[off:off + PB, u // 3, :],
                            pexpB2[off:off + PB, :, u], start=False, stop=True)
                    if g % 2:
                        nc.vector.tensor_scalar_max(hrelu[0:FC, u0:u0 + nun, :],
                                                    hps[0:FC, 0:nun, 0:BS], 0.0)
                    else:
                        nc.scalar.activation(hrelu[0:FC, u0:u0 + nun, :],
                                             hps[0:FC, 0:nun, 0:BS], AF.Relu)
                    for k in range(u0 // 4, (u0 + nun) // 4):
                        nc.tensor.matmul(
                            zps[0:128, 4 * k:4 * k + 4],
                            hrelu[0:FC + 1, 4 * k:4 * k + 4, :].rearrange(
                                "f u b -> f (u b)"),
                            w2s[:, k, :], start=True, stop=True)

            # head: full-width relu/mul/reduce; woute zero-masks the
            # off-diagonal strip lanes so they vanish in the reduce
            part = fcpool.tile([128, 1], f32)
            zr = fcpool.tile([128, U], f32)
            prod = fcpool.tile([128, U], f32)
            nc.vector.tensor_scalar_max(zr[:], zps[:], 0.0)
            nc.vector.tensor_mul(prod[:], zr[:], woute[:])
            nc.vector.tensor_reduce(part[:, 0:1], prod[:],
                                    axis=mybir.AxisListType.X, op=ALU.add)
            with tc.tile_pool(name="headps", bufs=1, space="PSUM") as hpsum:
                zf = hpsum.tile([1, BS], f32, tag="zf")
                nc.tensor.matmul(zf[0:1, :], part[:], Emat[:], start=True, stop=True)
                osb = fcpool.tile([1, BS], f32)
                nc.scalar.activation(osb[:], zf[0:1, :], AF.Sigmoid, bias=bout[0:1, :])
                nc.sync.dma_start(out_d[:], osb[:])

    nc.compile()
    return nc


def _prep_weights(i):
    """Host-side BN folding + layout. numpy fp32 math -> fp16 payloads."""
    f = lambda a: np.asarray(a, np.float32)
    w_conv, b_conv = f(i["w_conv"]), f(i["b_conv"])
    g1, be1, m1, v1 = f(i["g1"]), f(i["be1"]), f(i["m1"]), f(i["v1"])
    w_fc1, b_fc1 = f(i["w_fc1"]), f(i["b_fc1"])
    g2, be2, m2, v2 = f(i["g2"]), f(i["be2"]), f(i["m2"]), f(i["v2"])
    w_fc2, b_fc2 = f(i["w_fc2"]), f(i["b_fc2"])
    g3, be3, m3, v3 = f(i["g3"]), f(i["be3"]), f(i["m3"]), f(i["v3"])
    w_out, b_out = f(i["w_out"]), f(i["b_out"])

    s1 = g1 / np.sqrt(v1 + EPS)
    t1 = be1 - m1 * s1
    s2 = g2 / np.sqrt(v2 + EPS)
    b1pp = (b_fc1 - m2) * s2 + be2
    s3 = g3 / np.sqrt(v3 + EPS)
    w2pp = w_fc2 * s3[:, None]
    b2pp = (b_fc2 - m3) * s3 + be3

    # conv weights with BN1 scale folded; contraction index q = k*D + d
    Wc = np.ascontiguousarray(
        (w_conv * s1[:, None, None]).transpose(2, 1, 0).reshape(KD, U))
    # FC1 with BN2 scale and exp(t1 + s1*b_conv) folded
    gexp = np.exp(t1 + s1 * b_conv)
    w1pp = (w_fc1 * s2[:, :, None] * gexp[:, None, None]).transpose(2, 0, 1)  # (P,U,FC)
    w1a = np.empty((128, U, FC), np.float32)
    w1a[:PA] = w1pp[:PA]
    w1a[127] = b1pp                      # bias rides the const-1 pexp row
    # w1b: 3 units per partition group at offsets 0/32/64: [32*(u%3)+pb, u//3, f]
    w1b = np.zeros((64 + PB, FC, FC), np.float32)
    for u in range(U):
        w1b[32 * (u % 3):32 * (u % 3) + PB, u // 3] = w1pp[PA:P, u]

    # FC2 weights f-major with bias row: w2s[f, k, n] = w2pp[4k+n, f]
    w2s = np.empty((FC + 1, NG, 4), np.float32)
    w2s[:FC] = w2pp.T.reshape(FC, NG, 4)
    w2s[FC] = b2pp.reshape(NG, 4)

    # head: full-width, zero-masked: strip n rows keep only cols n::4
    woute = np.zeros((128, U), np.float32)
    for n in range(4):
        woute[32 * n:32 * n + 32, n::4] = w_out[n::4, 0][None]
    Em = np.zeros((128, BS), np.float32)
    for n in range(4):
        Em[32 * n:32 * n + 32] = np.eye(BS, dtype=np.float32)

    h16 = lambda a: np.asarray(a, np.float16)
    return {
        "wc": h16(Wc), "w1a": h16(w1a), "w1b": h16(w1b), "w2s": h16(w2s),
        "woute": woute, "Emat": Em,
        "onesrow": np.ones((1, BS * U), np.float16),
        "bout": np.asarray(b_out, np.float32).reshape(1, 1),
    }


def kernel(**inputs) -> np.ndarray:
    global _COMPILED
    if _COMPILED is None:
        _COMPILED = _build()
    nc = _COMPILED

    wmap = _prep_weights(inputs)
    x = np.asarray(inputs["input_seq"], np.float32)   # (256, 1000, 4)
    win = np.lib.stride_tricks.sliding_window_view(x, K, axis=1)  # (B, 982, D, K)
    in_maps = []
    for c in range(NCORES):
        xs = win[c * BS:(c + 1) * BS, :980]           # (32, 980, 4, 19)
        xcol = np.ascontiguousarray(
            xs.transpose(3, 2, 0, 1).astype(np.float16)).reshape(KD, BS, 980)
        tail = xcol[:, :, 7 * PA:].reshape(KD, 4, 8, PB, 7)
        xcolb = np.ascontiguousarray(tail.transpose(0, 1, 4, 2, 3)).reshape(KD, 4, 7, 104)
        in_maps.append({"xcol": xcol, "xcolb": xcolb, **wmap})

    res = run_bass_kernel_spmd(nc, in_maps, list(range(NCORES)))
    out = np.empty((B, 1), np.float32)
    for c in range(NCORES):
        out[c * BS:(c + 1) * BS, 0] = res.results[c]["out"][0]
    return out


# revision 21
# speedup vs baseline: 1.1307x; 1.0542x over previous
"""ExplaiNN Trainium2 kernel — 8-core SPMD, batch-sharded (32 rows/core).

Restructured from the 154.5us baseline (cost-model findings: DVE 106us busy
on fp32 reduce-max pooling + FC2; Pool engine 65us of SWDGE descriptor
generation; PE 41% busy). Now 113.4us with rel err 3.1e-3 (was 1.5e-2).

  dtype: fp16 matmul operands everywhere (vs fp32r/bf16) — halves DMA and
         keeps full-rate PE matmuls at any stream width, with better
         precision than bf16.
  conv:  X-stationary matmuls as before; per row j0-2 land in a 3-bank PSUM
         tile, j3-6 in a 4-bank tile. B-blocks (tail 13 pool windows x 8
         rows) run FIRST so their pexpB2 repack DMA chain hides under the
         A-row stream.
  pool:  only DVE and ACT can read PSUM (the Pool engine is DMA-only and a
         TensorTensor allows at most one PSUM input — hw limits). DVE
         reduce_max eats j0-2; one batched ACT copy moves j3-6 to fp16
         SBUF; DVE merges the 5 partials as a pair-max tree in fp16 2x
         mode, batched over 4 rows; batched ACT exp writes pexp. Merges are
         emitted one 4-row batch late and exps a batch later so they never
         park in the 4-deep wait queues and stall their sequencer (convoy
         avoidance); weight prefetch rides the otherwise-idle Pool queue.
  FC1:   weight-stationary per unit (ldweights w1 [128,100], stream pexp
         [128,32]) -> h lands f-major [100f, 32b] in PSUM, 16 units/bank,
         triple-buffered; relu alternates DVE/ACT -> hrelu fp16 (+const-1 row 100 carrying
         the FC2 bias). K-tail weights w1b packed 3 units per partition
         group at offsets 0/32/64 (matmul base-partition constraint), with
         pexpB2 replicated to those offsets.
  FC2:   per 4-unit group one PE pair: lhsT = hrelu [101, 4ux32b], rhs =
         w2 [101, 4] -> block-diagonal strips of zps [128, 300].
  head:  per strip: DVE relu, mul, reduce; partition mixdown via Emat
         matmul; ACT sigmoid; DMA out.
"""

import numpy as np
from contextlib import ExitStack

import concourse.bass as bass
import concourse.bacc as bacc
import concourse.mybir as mybir
import concourse.tile as tile
from concourse.bass_utils import run_bass_kernel_spmd

dt = mybir.dt

U, K, POOL, STRIDE, FC = 300, 19, 7, 7, 100
B, L, D = 256, 1000, 4
P = 140                     # pooled positions per row
EPS = 1e-5
NCORES = 8
BS = B // NCORES            # 32 rows per core
KD = K * D                  # 76 contraction
PA = 127                    # pool windows in the A-chunk (+1 const row = 128)
PB = P - PA                 # 13 windows in the B-chunk
CH = [(0, 300)]              # single conv pass
NG = U // 4                 # FC2 4-unit groups

_COMPILED = None


def _build():
    nc = bacc.Bacc("TRN2", target_bir_lowering=False, debug=False,
                   num_devices=NCORES)

    f16, f32 = dt.float16, dt.float32
    AF = mybir.ActivationFunctionType
    ALU = mybir.AluOpType

    xcol_d = nc.dram_tensor("xcol", [KD, BS, 980], f16, kind="ExternalInput").ap()
    xcolb_d = nc.dram_tensor("xcolb", [KD, 4, 7, 104], f16, kind="ExternalInput").ap()
    wc_d = nc.dram_tensor("wc", [KD, U], f16, kind="ExternalInput").ap()
    w1a_d = nc.dram_tensor("w1a", [128, U, FC], f16, kind="ExternalInput").ap()
    w1b_d = nc.dram_tensor("w1b", [64 + PB, FC, FC], f16, kind="ExternalInput").ap()
    w2_d = nc.dram_tensor("w2s", [FC + 1, NG, 4], f16, kind="ExternalInput").ap()
    wout_d = nc.dram_tensor("woute", [128, U], f32, kind="ExternalInput").ap()
    E_d = nc.dram_tensor("Emat", [128, BS], f32, kind="ExternalInput").ap()
    ones_d = nc.dram_tensor("onesrow", [1, BS * U], f16, kind="ExternalInput").ap()
    bout_d = nc.dram_tensor("bout", [1, 1], f32, kind="ExternalInput").ap()
    out_d = nc.dram_tensor("out", [1, BS], f32, kind="ExternalOutput").ap()

    with ExitStack() as ctx:
        tc = ctx.enter_context(tile.TileContext(nc))
        consts = ctx.enter_context(tc.tile_pool(name="consts", bufs=1))

        wc = consts.tile([KD, U], f16)
        xcolb = consts.tile([KD, 4, 7, 104], f16)
        w1a = consts.tile([128, U, FC], f16)
        w1b = consts.tile([64 + PB, FC, FC], f16)  # 3 units at partition 0/32/64
        w2s = consts.tile([FC + 1, NG, 4], f16)
        woute = consts.tile([128, U], f32)
        Emat = consts.tile([128, BS], f32)
        bout = consts.tile([1, 1], f32)
        pexp = consts.tile([128, BS, U], f16)       # [p(127)+const, b, u]
        pexpB = consts.tile([104, 4, U], f16)       # [(rr,pb), blk, u]
        pexpB2 = consts.tile([64 + PB, BS, U], f16)  # [pb, b, u] replicated @0/32/64

        nc.sync.dma_start(wc[:], wc_d[:])
        nc.sync.dma_start(xcolb[:, 0, :, :], xcolb_d[:, 0, :, :])
        nc.sync.dma_start(pexp[127:128, :, :].rearrange("p b u -> p (b u)"),
                            ones_d[:])

        # ---------------- conv + pool + exp ---------------------------------
        # Only DVE and ACT can read PSUM (Pool engine is DMA-only; a
        # TensorTensor may use at most one PSUM input). Per row: j0-2 land in
        # a 3-bank tile -> DVE reduce_max; j3-6 land in a 4-bank tile -> one
        # batched ACT copy to fp16 SBUF. The 5 partials merge on DVE as a
        # pair-max tree in fp16 2x mode, batched over 4 rows. Merges are
        # emitted one 4-row batch late and exps two batches late so neither
        # ever parks in the 4-deep wait queues and stalls its sequencer.
        pm = consts.tile([128, 36, U], f16)         # merged pool maxes
        with tc.tile_pool(name="xslab", bufs=3) as xpool, \
             tc.tile_pool(name="redps", bufs=1, space="PSUM") as rpsum, \
             tc.tile_pool(name="cpys", bufs=1, space="PSUM") as cpsum, \
             tc.tile_pool(name="qpool", bufs=2) as qpool, \
             tc.tile_pool(name="mpool", bufs=1) as mpool:


            for sq in range(30):
                nc.gpsimd.dma_start(w1a[:, 10 * sq:10 * sq + 10, :],
                                    w1a_d[:, 10 * sq:10 * sq + 10, :])
            nc.gpsimd.dma_start(w1b[:], w1b_d[:])
            nc.gpsimd.dma_start(w2s[:], w2_d[:])
            nc.gpsimd.dma_start(woute[:], wout_d[:])
            nc.gpsimd.dma_start(Emat[:], E_d[:])
            nc.gpsimd.dma_start(bout[:], bout_d[:])

            def conv_pool(lhs7, m, Q, C, qi):
                t3 = rpsum.tile([128, 3, 512], f32, tag="t3")
                for j in range(3):
                    nc.tensor.matmul(t3[0:m, j, 0:U], lhs7[j], wc[:],
                                     start=True, stop=True)
                t4 = cpsum.tile([128, 4, 512], f32, tag="t4")
                for j in range(3, 7):
                    nc.tensor.matmul(t4[0:m, j - 3, 0:U], lhs7[j], wc[:],
                                     start=True, stop=True)
                nc.vector.reduce_max(
                    Q[0:m, qi, :], t3[0:m, :, 0:U].rearrange("p j u -> p u j"),
                    axis=mybir.AxisListType.X)
                nc.scalar.activation(C[0:m, qi, :, :], t4[0:m, 0:4, 0:U], AF.Copy)

            def merges(Q, C, m, s0):
                m2 = mpool.tile([128, 4, 2, 300], f16, tag="m2")
                nc.vector.tensor_max(m2[0:m, :, :, :], C[0:m, :, 0:2, :],
                                     C[0:m, :, 2:4, :])
                m3 = mpool.tile([128, 4, 300], f16, tag="m3")
                nc.vector.tensor_max(m3[0:m, :, :], m2[0:m, :, 0, :],
                                     m2[0:m, :, 1, :])
                nc.vector.tensor_max(pm[0:m, s0:s0 + 4, :], m3[0:m, :, :],
                                     Q[0:m, :, :])

            batches = []     # (Q, C, m, pm-slot, exp destination)
            done_m, done_e = 0, 0

            def drain(upto_m, upto_e):
                nonlocal done_m, done_e
                while done_m < upto_m:
                    Q, C, m, s0, _ = batches[done_m]
                    merges(Q, C, m, s0)
                    done_m += 1
                while done_e < upto_e:
                    _, _, m, s0, dst = batches[done_e]
                    nc.scalar.activation(dst[:, :, 0:150], pm[0:m, s0:s0 + 4, 0:150], AF.Exp)
                    nc.scalar.activation(dst[:, :, 150:U], pm[0:m, s0:s0 + 4, 150:U], AF.Exp)
                    done_e += 1

            # B blocks first so their repack DMAs hide under the A rows
            slab0 = xpool.tile([KD, 4, 980], f16, tag="slab")
            nc.sync.dma_start(slab0[:], xcol_d[:, 0:4, :])
            for bq in range(1, 4):
                nc.sync.dma_start(xcolb[:, bq, :, :], xcolb_d[:, bq, :, :])
            Q = qpool.tile([128, 4, 300], f16, tag="Q")
            C = qpool.tile([128, 4, 4, 300], f16, tag="C")
            for blk in range(4):
                lhs7 = [xcolb[:, blk, j, :] for j in range(7)]
                conv_pool(lhs7, 8 * PB, Q, C, blk)
            batches.append((Q, C, 8 * PB, 32, pexpB[0:8 * PB, 0:4, :]))
            for sb in range(8):
                if sb == 0:
                    slab = slab0
                else:
                    slab = xpool.tile([KD, 4, 980], f16, tag="slab")
                    nc.sync.dma_start(slab[:], xcol_d[:, 4 * sb:4 * sb + 4, :])
                Q = qpool.tile([128, 4, 300], f16, tag="Q")
                C = qpool.tile([128, 4, 4, 300], f16, tag="C")
                slabr = slab[:].rearrange("q r (p j) -> q r p j", j=7)
                for r in range(4):
                    lhs7 = [slabr[:, r, 0:PA, j] for j in range(7)]
                    conv_pool(lhs7, PA, Q, C, r)
                    if r == 3:
                        drain(sb + 1, sb + 1)
                if sb == 1:
                    # repack pexpB[(rr,pb), blk, u] -> pexpB2[pb, 8*blk+rr, u]
                    for rr in range(8):
                        nc.sync.dma_start(pexpB2[0:PB, rr:BS:8, :],
                                          pexpB[rr * PB:(rr + 1) * PB, :, :])
                    for off in (32, 64):
                        nc.sync.dma_start(pexpB2[off:off + PB, :, :],
                                          pexpB2[0:PB, :, :])
                batches.append((Q, C, PA, 4 * sb,
                                pexp[0:PA, 4 * sb:4 * sb + 4, :]))
            drain(9, 9)

        # ---------------- FC1 + relu + FC2 + head ---------------------------
        fcpool = ctx.enter_context(tc.tile_pool(name="fcsb", bufs=1))
        hrelu = fcpool.tile([FC + 1, U, BS], f16)   # [f+const, u, b]
        nc.sync.dma_start(hrelu[FC:FC + 1, :, :].rearrange("p u b -> p (u b)"),
                            ones_d[:])
        zps_pool = ctx.enter_context(tc.tile_pool(name="zpsp", bufs=1, space="PSUM"))
        zps = zps_pool.tile([128, U], f32)

        with tc.tile_pool(name="fcps", bufs=3, space="PSUM") as fpsum:
            for ci, (c0, w) in enumerate(CH):
                for g in range((w + 15) // 16):
                    u0 = c0 + 16 * g
                    nun = min(16, c0 + w - u0)
                    hps = fpsum.tile([FC, 16, BS], f32, tag="hps")
                    for s in range(nun):
                        u = u0 + s
                        o = hps[0:FC, s, 0:BS]
                        nc.tensor.matmul(o, w1a[:, u, :], pexp[:, :, u],
                                         start=True, stop=False)
                        off = 32 * (u % 3)
                        nc.tensor.matmul(
                            o, w1b[off:off + PB, u // 3, :],
                            pexpB2[off:off + PB, :, u], start=False, stop=True)
                    if g % 2:
                        nc.vector.tensor_scalar_max(hrelu[0:FC, u0:u0 + nun, :],
                                                    hps[0:FC, 0:nun, 0:BS], 0.0)
                    else:
                        nc.scalar.activation(hrelu[0:FC, u0:u0 + nun, :],
                                             hps[0:FC, 0:nun, 0:BS], AF.Relu)
                    for k in range(u0 // 4, (u0 + nun) // 4):
                        nc.tensor.matmul(
                            zps[0:128, 4 * k:4 * k + 4],
                            hrelu[0:FC + 1, 4 * k:4 * k + 4, :].rearrange(
                                "f u b -> f (u b)"),
                            w2s[:, k, :], start=True, stop=True)

            # head: full-width relu/mul/reduce; woute zero-masks the
            # off-diagonal strip lanes so they vanish in the reduce
            part = fcpool.tile([128, 1], f32)
            zr = fcpool.tile([128, U], f32)
            prod = fcpool.tile([128, U], f32)
            nc.vector.tensor_scalar_max(zr[:], zps[:], 0.0)
            nc.vector.tensor_mul(prod[:], zr[:], woute[:])
            nc.vector.tensor_reduce(part[:, 0:1], prod[:],
                                    axis=mybir.AxisListType.X, op=ALU.add)
            with tc.tile_pool(name="headps", bufs=1, space="PSUM") as hpsum:
                zf = hpsum.tile([1, BS], f32, tag="zf")
                nc.tensor.matmul(zf[0:1, :], part[:], Emat[:], start=True, stop=True)
                osb = fcpool.tile([1, BS], f32)
                nc.scalar.activation(osb[:], zf[0:1, :], AF.Sigmoid, bias=bout[0:1, :])
                nc.sync.dma_start(out_d[:], osb[:])

    nc.compile()
    return nc


def _prep_weights(i):
    """Host-side BN folding + layout. numpy fp32 math -> fp16 payloads."""
    f = lambda a: np.asarray(a, np.float32)
    w_conv, b_conv = f(i["w_conv"]), f(i["b_conv"])
    g1, be1, m1, v1 = f(i["g1"]), f(i["be1"]), f(i["m1"]), f(i["v1"])
    w_fc1, b_fc1 = f(i["w_fc1"]), f(i["b_fc1"])
    g2, be2, m2, v2 = f(i["g2"]), f(i["be2"]), f(i["m2"]), f(i["v2"])
    w_fc2, b_fc2 = f(i["w_fc2"]), f(i["b_fc2"])
    g3, be3, m3, v3 = f(i["g3"]), f(i["be3"]), f(i["m3"]), f(i["v3"])
    w_out, b_out = f(i["w_out"]), f(i["b_out"])

    s1 = g1 / np.sqrt(v1 + EPS)
    t1 = be1 - m1 * s1
    s2 = g2 / np.sqrt(v2 + EPS)
    b1pp = (b_fc1 - m2) * s2 + be2
    s3 = g3 / np.sqrt(v3 + EPS)
    w2pp = w_fc2 * s3[:, None]
    b2pp = (b_fc2 - m3) * s3 + be3

    # conv weights with BN1 scale folded; contraction index q = k*D + d
    Wc = np.ascontiguousarray(
        (w_conv * s1[:, None, None]).transpose(2, 1, 0).reshape(KD, U))
    # FC1 with BN2 scale and exp(t1 + s1*b_conv) folded
    gexp = np.exp(t1 + s1 * b_conv)
    w1pp = (w_fc1 * s2[:, :, None] * gexp[:, None, None]).transpose(2, 0, 1)  # (P,U,FC)
    w1a = np.empty((128, U, FC), np.float32)
    w1a[:PA] = w1pp[:PA]
    w1a[127] = b1pp                      # bias rides the const-1 pexp row
    # w1b: 3 units per partition group at offsets 0/32/64: [32*(u%3)+pb, u//3, f]
    w1b = np.zeros((64 + PB, FC, FC), np.float32)
    for u in range(U):
        w1b[32 * (u % 3):32 * (u % 3) + PB, u // 3] = w1pp[PA:P, u]

    # FC2 weights f-major with bias row: w2s[f, k, n] = w2pp[4k+n, f]
    w2s = np.empty((FC + 1, NG, 4), np.float32)
    w2s[:FC] = w2pp.T.reshape(FC, NG, 4)
    w2s[FC] = b2pp.reshape(NG, 4)

    # head: full-width, zero-masked: strip n rows keep only cols n::4
    woute = np.zeros((128, U), np.float32)
    for n in range(4):
        woute[32 * n:32 * n + 32, n::4] = w_out[n::4, 0][None]
    Em = np.zeros((128, BS), np.float32)
    for n in range(4):
        Em[32 * n:32 * n + 32] = np.eye(BS, dtype=np.float32)

    h16 = lambda a: np.asarray(a, np.float16)
    return {
        "wc": h16(Wc), "w1a": h16(w1a), "w1b": h16(w1b), "w2s": h16(w2s),
        "woute": woute, "Emat": Em,
        "onesrow": np.ones((1, BS * U), np.float16),
        "bout": np.asarray(b_out, np.float32).reshape(1, 1),
    }


def kernel(**inputs) -> np.ndarray:
    global _COMPILED
    if _COMPILED is None:
        _COMPILED = _build()
    nc = _COMPILED

    wmap = _prep_weights(inputs)
    x = np.asarray(inputs["input_seq"], np.float32)   # (256, 1000, 4)
    win = np.lib.stride_tricks.sliding_window_view(x, K, axis=1)  # (B, 982, D, K)
    in_maps = []
    for c in range(NCORES):
        xs = win[c * BS:(c + 1) * BS, :980]           # (32, 980, 4, 19)
        xcol = np.ascontiguousarray(
            xs.transpose(3, 2, 0, 1).astype(np.float16)).reshape(KD, BS, 980)
        tail = xcol[:, :, 7 * PA:].reshape(KD, 4, 8, PB, 7)
        xcolb = np.ascontiguousarray(tail.transpose(0, 1, 4, 2, 3)).reshape(KD, 4, 7, 104)
        in_maps.append({"xcol": xcol, "xcolb": xcolb, **wmap})

    res = run_bass_kernel_spmd(nc, in_maps, list(range(NCORES)))
    out = np.empty((B, 1), np.float32)
    for c in range(NCORES):
        out[c * BS:(c + 1) * BS, 0] = res.results[c]["out"][0]
    return out


# revision 24
# speedup vs baseline: 1.1312x; 1.0005x over previous
"""ExplaiNN Trainium2 kernel — 8-core SPMD, batch-sharded (32 rows/core).

Restructured from the 154.5us baseline (cost-model findings: DVE 106us busy
on fp32 reduce-max pooling + FC2; Pool engine 65us of SWDGE descriptor
generation; PE 41% busy). Now 113.4us with rel err 3.1e-3 (was 1.5e-2).

  dtype: fp16 matmul operands everywhere (vs fp32r/bf16) — halves DMA and
         keeps full-rate PE matmuls at any stream width, with better
         precision than bf16.
  conv:  X-stationary matmuls as before; per row j0-2 land in a 3-bank PSUM
         tile, j3-6 in a 4-bank tile. B-blocks (tail 13 pool windows x 8
         rows) run FIRST so their pexpB2 repack DMA chain hides under the
         A-row stream.
  pool:  only DVE and ACT can read PSUM (the Pool engine is DMA-only and a
         TensorTensor allows at most one PSUM input — hw limits). DVE
         reduce_max eats j0-2; one batched ACT copy moves j3-6 to fp16
         SBUF; DVE merges the 5 partials as a pair-max tree in fp16 2x
         mode, batched over 4 rows; batched ACT exp writes pexp. Merges are
         emitted one 4-row batch late and exps a batch later so they never
         park in the 4-deep wait queues and stall their sequencer (convoy
         avoidance); weight prefetch rides the otherwise-idle Pool queue.
  FC1:   weight-stationary per unit (ldweights w1 [128,100], stream pexp
         [128,32]) -> h lands f-major [100f, 32b] in PSUM, 16 units/bank,
         triple-buffered; relu alternates DVE/ACT -> hrelu fp16 (+const-1 row 100 carrying
         the FC2 bias). K-tail weights w1b packed 3 units per partition
         group at offsets 0/32/64 (matmul base-partition constraint), with
         pexpB2 replicated to those offsets.
  FC2:   per 4-unit group one PE pair: lhsT = hrelu [101, 4ux32b], rhs =
         w2 [101, 4] -> block-diagonal strips of zps [128, 300].
  head:  per strip: DVE relu, mul, reduce; partition mixdown via Emat
         matmul; ACT sigmoid; DMA out.
"""

import numpy as np
from contextlib import ExitStack

import concourse.bass as bass
import concourse.bacc as bacc
import concourse.mybir as mybir
import concourse.tile as tile
from concourse.bass_utils import run_bass_kernel_spmd

dt = mybir.dt

U, K, POOL, STRIDE, FC = 300, 19, 7, 7, 100
B, L, D = 256, 1000, 4
P = 140                     # pooled positions per row
EPS = 1e-5
NCORES = 8
BS = B // NCORES            # 32 rows per core
KD = K * D                  # 76 contraction
PA = 127                    # pool windows in the A-chunk (+1 const row = 128)
PB = P - PA                 # 13 windows in the B-chunk
CH = [(0, 300)]              # single conv pass
NG = U // 4                 # FC2 4-unit groups

_COMPILED = None


def _build():
    nc = bacc.Bacc("TRN2", target_bir_lowering=False, debug=False,
                   num_devices=NCORES)

    f16, f32 = dt.float16, dt.float32
    AF = mybir.ActivationFunctionType
    ALU = mybir.AluOpType

    xcol_d = nc.dram_tensor("xcol", [KD, BS, 980], f16, kind="ExternalInput").ap()
    xcolb_d = nc.dram_tensor("xcolb", [KD, 4, 7, 104], f16, kind="ExternalInput").ap()
    wc_d = nc.dram_tensor("wc", [KD, U], f16, kind="ExternalInput").ap()
    w1a_d = nc.dram_tensor("w1a", [128, U, FC], f16, kind="ExternalInput").ap()
    w1b_d = nc.dram_tensor("w1b", [64 + PB, FC, FC], f16, kind="ExternalInput").ap()
    w2_d = nc.dram_tensor("w2s", [FC + 1, NG, 4], f16, kind="ExternalInput").ap()
    wout_d = nc.dram_tensor("woute", [128, U], f32, kind="ExternalInput").ap()
    E_d = nc.dram_tensor("Emat", [128, BS], f32, kind="ExternalInput").ap()
    ones_d = nc.dram_tensor("onesrow", [1, BS * U], f16, kind="ExternalInput").ap()
    bout_d = nc.dram_tensor("bout", [1, 1], f32, kind="ExternalInput").ap()
    out_d = nc.dram_tensor("out", [1, BS], f32, kind="ExternalOutput").ap()

    with ExitStack() as ctx:
        tc = ctx.enter_context(tile.TileContext(nc))
        consts = ctx.enter_context(tc.tile_pool(name="consts", bufs=1))

        wc = consts.tile([KD, U], f16)
        xcolb = consts.tile([KD, 4, 7, 104], f16)
        w1a = consts.tile([128, U, FC], f16)
        w1b = consts.tile([64 + PB, FC, FC], f16)  # 3 units at partition 0/32/64
        w2s = consts.tile([FC + 1, NG, 4], f16)
        woute = consts.tile([128, U], f32)
        Emat = consts.tile([128, BS], f32)
        bout = consts.tile([1, 1], f32)
        pexp = consts.tile([128, BS, U], f16)       # [p(127)+const, b, u]
        pexpB = consts.tile([104, 4, U], f16)       # [(rr,pb), blk, u]
        pexpB2 = consts.tile([64 + PB, BS, U], f16)  # [pb, b, u] replicated @0/32/64

        nc.sync.dma_start(wc[:], wc_d[:])
        nc.sync.dma_start(xcolb[:, 0, :, :], xcolb_d[:, 0, :, :])
        nc.sync.dma_start(pexp[127:128, :, :].rearrange("p b u -> p (b u)"),
                            ones_d[:])

        # ---------------- conv + pool + exp ---------------------------------
        # Only DVE and ACT can read PSUM (Pool engine is DMA-only; a
        # TensorTensor may use at most one PSUM input). Per row: j0-2 land in
        # a 3-bank tile -> DVE reduce_max; j3-6 land in a 4-bank tile -> one
        # batched ACT copy to fp16 SBUF. The 5 partials merge on DVE as a
        # pair-max tree in fp16 2x mode, batched over 4 rows. Merges are
        # emitted one 4-row batch late and exps two batches late so neither
        # ever parks in the 4-deep wait queues and stalls its sequencer.
        pm = consts.tile([128, 36, U], f16)         # merged pool maxes
        with tc.tile_pool(name="xslab", bufs=3) as xpool, \
             tc.tile_pool(name="redps", bufs=1, space="PSUM") as rpsum, \
             tc.tile_pool(name="cpys", bufs=1, space="PSUM") as cpsum, \
             tc.tile_pool(name="qpool", bufs=2) as qpool, \
             tc.tile_pool(name="mpool", bufs=1) as mpool:


            for sq in range(30):
                nc.gpsimd.dma_start(w1a[:, 10 * sq:10 * sq + 10, :],
                                    w1a_d[:, 10 * sq:10 * sq + 10, :])
            nc.gpsimd.dma_start(w1b[:], w1b_d[:])
            nc.gpsimd.dma_start(w2s[:], w2_d[:])
            nc.gpsimd.dma_start(woute[:], wout_d[:])
            nc.gpsimd.dma_start(Emat[:], E_d[:])
            nc.gpsimd.dma_start(bout[:], bout_d[:])

            def conv_pool(lhs7, m, Q, C, qi):
                t3 = rpsum.tile([128, 3, 512], f32, tag="t3")
                for j in range(3):
                    nc.tensor.matmul(t3[0:m, j, 0:U], lhs7[j], wc[:],
                                     start=True, stop=True)
                t4 = cpsum.tile([128, 4, 512], f32, tag="t4")
                for j in range(3, 7):
                    nc.tensor.matmul(t4[0:m, j - 3, 0:U], lhs7[j], wc[:],
                                     start=True, stop=True)
                nc.vector.reduce_max(
                    Q[0:m, qi, :], t3[0:m, :, 0:U].rearrange("p j u -> p u j"),
                    axis=mybir.AxisListType.X)
                nc.scalar.activation(C[0:m, qi, :, :], t4[0:m, 0:4, 0:U], AF.Copy)

            def merges(Q, C, m, s0):
                m2 = mpool.tile([128, 4, 2, 300], f16, tag="m2")
                nc.vector.tensor_max(m2[0:m, :, :, :], C[0:m, :, 0:2, :],
                                     C[0:m, :, 2:4, :])
                m3 = mpool.tile([128, 4, 300], f16, tag="m3")
                nc.vector.tensor_max(m3[0:m, :, :], m2[0:m, :, 0, :],
                                     m2[0:m, :, 1, :])
                nc.vector.tensor_max(pm[0:m, s0:s0 + 4, :], m3[0:m, :, :],
                                     Q[0:m, :, :])

            batches = []     # (Q, C, m, pm-slot, exp destination)
            done_m, done_e = 0, 0

            def drain(upto_m, upto_e):
                nonlocal done_m, done_e
                while done_m < upto_m:
                    Q, C, m, s0, _ = batches[done_m]
                    merges(Q, C, m, s0)
                    done_m += 1
                while done_e < upto_e:
                    _, _, m, s0, dst = batches[done_e]
                    nc.scalar.activation(dst[:, :, 0:150], pm[0:m, s0:s0 + 4, 0:150], AF.Exp)
                    nc.scalar.activation(dst[:, :, 150:U], pm[0:m, s0:s0 + 4, 150:U], AF.Exp)
                    done_e += 1

            # B blocks first so their repack DMAs hide under the A rows
            slab0 = xpool.tile([KD, 4, 980], f16, tag="slab")
            nc.sync.dma_start(slab0[:, 0:2, :], xcol_d[:, 0:2, :])
            nc.sync.dma_start(slab0[:, 2:4, :], xcol_d[:, 2:4, :])
            for bq in range(1, 4):
                nc.sync.dma_start(xcolb[:, bq, :, :], xcolb_d[:, bq, :, :])
            Q = qpool.tile([128, 4, 300], f16, tag="Q")
            C = qpool.tile([128, 4, 4, 300], f16, tag="C")
            for blk in range(4):
                lhs7 = [xcolb[:, blk, j, :] for j in range(7)]
                conv_pool(lhs7, 8 * PB, Q, C, blk)
            batches.append((Q, C, 8 * PB, 32, pexpB[0:8 * PB, 0:4, :]))
            for sb in range(8):
                if sb == 0:
                    slab = slab0
                else:
                    slab = xpool.tile([KD, 4, 980], f16, tag="slab")
                    nc.sync.dma_start(slab[:], xcol_d[:, 4 * sb:4 * sb + 4, :])
                Q = qpool.tile([128, 4, 300], f16, tag="Q")
                C = qpool.tile([128, 4, 4, 300], f16, tag="C")
                slabr = slab[:].rearrange("q r (p j) -> q r p j", j=7)
                for r in range(4):
                    lhs7 = [slabr[:, r, 0:PA, j] for j in range(7)]
                    conv_pool(lhs7, PA, Q, C, r)
                    if r == 3:
                        drain(sb + 1, sb + 1)
                if sb == 1:
                    # repack pexpB[(rr,pb), blk, u] -> pexpB2[pb, 8*blk+rr, u]
                    for rr in range(8):
                        nc.sync.dma_start(pexpB2[0:PB, rr:BS:8, :],
                                          pexpB[rr * PB:(rr + 1) * PB, :, :])
                    for off in (32, 64):
                        nc.sync.dma_start(pexpB2[off:off + PB, :, :],
                                          pexpB2[0:PB, :, :])
                batches.append((Q, C, PA, 4 * sb,
                                pexp[0:PA, 4 * sb:4 * sb + 4, :]))
            drain(9, 9)

        # ---------------- FC1 + relu + FC2 + head ---------------------------
        fcpool = ctx.enter_context(tc.tile_pool(name="fcsb", bufs=1))
        hrelu = fcpool.tile([FC + 1, U, BS], f16)   # [f+const, u, b]
        nc.sync.dma_start(hrelu[FC:FC + 1, :, :].rearrange("p u b -> p (u b)"),
                            ones_d[:])
        zps_pool = ctx.enter_context(tc.tile_pool(name="zpsp", bufs=1, space="PSUM"))
        zps = zps_pool.tile([128, U], f32)

        with tc.tile_pool(name="fcps", bufs=3, space="PSUM") as fpsum:
            for ci, (c0, w) in enumerate(CH):
                for g in range((w + 15) // 16):
                    u0 = c0 + 16 * g
                    nun = min(16, c0 + w - u0)
                    hps = fpsum.tile([FC, 16, BS], f32, tag="hps")
                    for s in range(nun):
                        u = u0 + s
                        o = hps[0:FC, s, 0:BS]
                        nc.tensor.matmul(o, w1a[:, u, :], pexp[:, :, u],
                                         start=True, stop=False)
                        off = 32 * (u % 3)
                        nc.tensor.matmul(
                            o, w1b[off:off + PB, u // 3, :],
                            pexpB2[off:off + PB, :, u], start=False, stop=True)
                    if g % 2:
                        nc.vector.tensor_scalar_max(hrelu[0:FC, u0:u0 + nun, :],
                                                    hps[0:FC, 0:nun, 0:BS], 0.0)
                    else:
                        nc.scalar.activation(hrelu[0:FC, u0:u0 + nun, :],
                                             hps[0:FC, 0:nun, 0:BS], AF.Relu)
                    for k in range(u0 // 4, (u0 + nun) // 4):
                        nc.tensor.matmul(
                            zps[0:128, 4 * k:4 * k + 4],
                            hrelu[0:FC + 1, 4 * k:4 * k + 4, :].rearrange(
                                "f u b -> f (u b)"),
                            w2s[:, k, :], start=True, stop=True)

            # head: full-width relu/mul/reduce; woute zero-masks the
            # off-diagonal strip lanes so they vanish in the reduce
            part = fcpool.tile([128, 1], f32)
            zr = fcpool.tile([128, U], f32)
            prod = fcpool.tile([128, U], f32)
            nc.vector.tensor_scalar_max(zr[:], zps[:], 0.0)
            nc.vector.tensor_mul(prod[:], zr[:], woute[:])
            nc.vector.tensor_reduce(part[:, 0:1], prod[:],
                                    axis=mybir.AxisListType.X, op=ALU.add)
            with tc.tile_pool(name="headps", bufs=1, space="PSUM") as hpsum:
                zf = hpsum.tile([1, BS], f32, tag="zf")
                nc.tensor.matmul(zf[0:1, :], part[:], Emat[:], start=True, stop=True)
                osb = fcpool.tile([1, BS], f32)
                nc.scalar.activation(osb[:], zf[0:1, :], AF.Sigmoid, bias=bout[0:1, :])
                nc.sync.dma_start(out_d[:], osb[:])

    nc.compile()
    return nc


def _prep_weights(i):
    """Host-side BN folding + layout. numpy fp32 math -> fp16 payloads."""
    f = lambda a: np.asarray(a, np.float32)
    w_conv, b_conv = f(i["w_conv"]), f(i["b_conv"])
    g1, be1, m1, v1 = f(i["g1"]), f(i["be1"]), f(i["m1"]), f(i["v1"])
    w_fc1, b_fc1 = f(i["w_fc1"]), f(i["b_fc1"])
    g2, be2, m2, v2 = f(i["g2"]), f(i["be2"]), f(i["m2"]), f(i["v2"])
    w_fc2, b_fc2 = f(i["w_fc2"]), f(i["b_fc2"])
    g3, be3, m3, v3 = f(i["g3"]), f(i["be3"]), f(i["m3"]), f(i["v3"])
    w_out, b_out = f(i["w_out"]), f(i["b_out"])

    s1 = g1 / np.sqrt(v1 + EPS)
    t1 = be1 - m1 * s1
    s2 = g2 / np.sqrt(v2 + EPS)
    b1pp = (b_fc1 - m2) * s2 + be2
    s3 = g3 / np.sqrt(v3 + EPS)
    w2pp = w_fc2 * s3[:, None]
    b2pp = (b_fc2 - m3) * s3 + be3

    # conv weights with BN1 scale folded; contraction index q = k*D + d
    Wc = np.ascontiguousarray(
        (w_conv * s1[:, None, None]).transpose(2, 1, 0).reshape(KD, U))
    # FC1 with BN2 scale and exp(t1 + s1*b_conv) folded
    gexp = np.exp(t1 + s1 * b_conv)
    w1pp = (w_fc1 * s2[:, :, None] * gexp[:, None, None]).transpose(2, 0, 1)  # (P,U,FC)
    w1a = np.empty((128, U, FC), np.float32)
    w1a[:PA] = w1pp[:PA]
    w1a[127] = b1pp                      # bias rides the const-1 pexp row
    # w1b: 3 units per partition group at offsets 0/32/64: [32*(u%3)+pb, u//3, f]
    w1b = np.zeros((64 + PB, FC, FC), np.float32)
    for u in range(U):
        w1b[32 * (u % 3):32 * (u % 3) + PB, u // 3] = w1pp[PA:P, u]

    # FC2 weights f-major with bias row: w2s[f, k, n] = w2pp[4k+n, f]
    w2s = np.empty((FC + 1, NG, 4), np.float32)
    w2s[:FC] = w2pp.T.reshape(FC, NG, 4)
    w2s[FC] = b2pp.reshape(NG, 4)

    # head: full-width, zero-masked: strip n rows keep only cols n::4
    woute = np.zeros((128, U), np.float32)
    for n in range(4):
        woute[32 * n:32 * n + 32, n::4] = w_out[n::4, 0][None]
    Em = np.zeros((128, BS), np.float32)
    for n in range(4):
        Em[32 * n:32 * n + 32] = np.eye(BS, dtype=np.float32)

    h16 = lambda a: np.asarray(a, np.float16)
    return {
        "wc": h16(Wc), "w1a": h16(w1a), "w1b": h16(w1b), "w2s": h16(w2s),
        "woute": woute, "Emat": Em,
        "onesrow": np.ones((1, BS * U), np.float16),
        "bout": np.asarray(b_out, np.float32).reshape(1, 1),
    }


def kernel(**inputs) -> np.ndarray:
    global _COMPILED
    if _COMPILED is None:
        _COMPILED = _build()
    nc = _COMPILED

    wmap = _prep_weights(inputs)
    x = np.asarray(inputs["input_seq"], np.float32)   # (256, 1000, 4)
    win = np.lib.stride_tricks.sliding_window_view(x, K, axis=1)  # (B, 982, D, K)
    in_maps = []
    for c in range(NCORES):
        xs = win[c * BS:(c + 1) * BS, :980]           # (32, 980, 4, 19)
        xcol = np.ascontiguousarray(
            xs.transpose(3, 2, 0, 1).astype(np.float16)).reshape(KD, BS, 980)
        tail = xcol[:, :, 7 * PA:].reshape(KD, 4, 8, PB, 7)
        xcolb = np.ascontiguousarray(tail.transpose(0, 1, 4, 2, 3)).reshape(KD, 4, 7, 104)
        in_maps.append({"xcol": xcol, "xcolb": xcolb, **wmap})

    res = run_bass_kernel_spmd(nc, in_maps, list(range(NCORES)))
    out = np.empty((B, 1), np.float32)
    for c in range(NCORES):
        out[c * BS:(c + 1) * BS, 0] = res.results[c]["out"][0]
    return out


# revision 25
# speedup vs baseline: 1.1464x; 1.0134x over previous
"""ExplaiNN Trainium2 kernel — 8-core SPMD, batch-sharded (32 rows/core).

Restructured from the 154.5us baseline (cost-model findings: DVE 106us busy
on fp32 reduce-max pooling + FC2; Pool engine 65us of SWDGE descriptor
generation; PE 41% busy). Now 113.4us with rel err 3.1e-3 (was 1.5e-2).

  dtype: fp16 matmul operands everywhere (vs fp32r/bf16) — halves DMA and
         keeps full-rate PE matmuls at any stream width, with better
         precision than bf16.
  conv:  X-stationary matmuls as before; per row j0-2 land in a 3-bank PSUM
         tile, j3-6 in a 4-bank tile. B-blocks (tail 13 pool windows x 8
         rows) run FIRST so their pexpB2 repack DMA chain hides under the
         A-row stream.
  pool:  only DVE and ACT can read PSUM (the Pool engine is DMA-only and a
         TensorTensor allows at most one PSUM input — hw limits). DVE
         reduce_max eats j0-2; one batched ACT copy moves j3-6 to fp16
         SBUF; DVE merges the 5 partials as a pair-max tree in fp16 2x
         mode, batched over 4 rows; batched ACT exp writes pexp. Merges are
         emitted one 4-row batch late and exps a batch later so they never
         park in the 4-deep wait queues and stall their sequencer (convoy
         avoidance); weight prefetch rides the otherwise-idle Pool queue.
  FC1:   weight-stationary per unit (ldweights w1 [128,100], stream pexp
         [128,32]) -> h lands f-major [100f, 32b] in PSUM, 16 units/bank,
         triple-buffered; relu alternates DVE/ACT -> hrelu fp16 (+const-1 row 100 carrying
         the FC2 bias). K-tail weights w1b packed 3 units per partition
         group at offsets 0/32/64 (matmul base-partition constraint), with
         pexpB2 replicated to those offsets.
  FC2:   per 4-unit group one PE pair: lhsT = hrelu [101, 4ux32b], rhs =
         w2 [101, 4] -> block-diagonal strips of zps [128, 300].
  head:  per strip: DVE relu, mul, reduce; partition mixdown via Emat
         matmul; ACT sigmoid; DMA out.
"""

import numpy as np
from contextlib import ExitStack

import concourse.bass as bass
import concourse.bacc as bacc
import concourse.mybir as mybir
import concourse.tile as tile
from concourse.bass_utils import run_bass_kernel_spmd

dt = mybir.dt

U, K, POOL, STRIDE, FC = 300, 19, 7, 7, 100
B, L, D = 256, 1000, 4
P = 140                     # pooled positions per row
EPS = 1e-5
NCORES = 8
BS = B // NCORES            # 32 rows per core
KD = K * D                  # 76 contraction
PA = 127                    # pool windows in the A-chunk (+1 const row = 128)
PB = P - PA                 # 13 windows in the B-chunk
CH = [(0, 300)]              # single conv pass
NG = U // 4                 # FC2 4-unit groups

_COMPILED = None


def _build():
    nc = bacc.Bacc("TRN2", target_bir_lowering=False, debug=False,
                   num_devices=NCORES)

    f16, f32 = dt.float16, dt.float32
    AF = mybir.ActivationFunctionType
    ALU = mybir.AluOpType

    xcol_d = nc.dram_tensor("xcol", [KD, BS, 980], f16, kind="ExternalInput").ap()
    xcolb_d = nc.dram_tensor("xcolb", [KD, 4, 7, 104], f16, kind="ExternalInput").ap()
    wc_d = nc.dram_tensor("wc", [KD, U], f16, kind="ExternalInput").ap()
    w1a_d = nc.dram_tensor("w1a", [128, U, FC], f16, kind="ExternalInput").ap()
    w1b_d = nc.dram_tensor("w1b", [64 + PB, FC, FC], f16, kind="ExternalInput").ap()
    w2_d = nc.dram_tensor("w2s", [FC + 1, NG, 4], f16, kind="ExternalInput").ap()
    wout_d = nc.dram_tensor("woute", [128, U], f32, kind="ExternalInput").ap()
    E_d = nc.dram_tensor("Emat", [128, BS], f32, kind="ExternalInput").ap()
    ones_d = nc.dram_tensor("onesrow", [1, BS * U], f16, kind="ExternalInput").ap()
    bout_d = nc.dram_tensor("bout", [1, 1], f32, kind="ExternalInput").ap()
    out_d = nc.dram_tensor("out", [1, BS], f32, kind="ExternalOutput").ap()

    with ExitStack() as ctx:
        tc = ctx.enter_context(tile.TileContext(nc))
        consts = ctx.enter_context(tc.tile_pool(name="consts", bufs=1))

        wc = consts.tile([KD, U], f16)
        xcolb = consts.tile([KD, 4, 7, 104], f16)
        w1a = consts.tile([128, U, FC], f16)
        w1b = consts.tile([64 + PB, FC, FC], f16)  # 3 units at partition 0/32/64
        w2s = consts.tile([FC + 1, NG, 4], f16)
        woute = consts.tile([128, U], f32)
        Emat = consts.tile([128, BS], f32)
        bout = consts.tile([1, 1], f32)
        pexp = consts.tile([128, BS, U], f16)       # [p(127)+const, b, u]
        pexpB = consts.tile([104, 4, U], f16)       # [(rr,pb), blk, u]
        pexpB2 = consts.tile([64 + PB, BS, U], f16)  # [pb, b, u] replicated @0/32/64

        nc.sync.dma_start(wc[:], wc_d[:])
        nc.sync.dma_start(xcolb[:, 0, :, :], xcolb_d[:, 0, :, :])
        nc.sync.dma_start(pexp[127:128, :, :].rearrange("p b u -> p (b u)"),
                            ones_d[:])

        # ---------------- conv + pool + exp ---------------------------------
        # Only DVE and ACT can read PSUM (Pool engine is DMA-only; a
        # TensorTensor may use at most one PSUM input). Per row: j0-2 land in
        # a 3-bank tile -> DVE reduce_max; j3-6 land in a 4-bank tile -> one
        # batched ACT copy to fp16 SBUF. The 5 partials merge on DVE as a
        # pair-max tree in fp16 2x mode, batched over 4 rows. Merges are
        # emitted one 4-row batch late and exps two batches late so neither
        # ever parks in the 4-deep wait queues and stalls its sequencer.
        pm = consts.tile([128, 36, U], f16)         # merged pool maxes
        with tc.tile_pool(name="xslab", bufs=3) as xpool, \
             tc.tile_pool(name="redps", bufs=1, space="PSUM") as rpsum, \
             tc.tile_pool(name="cpys", bufs=1, space="PSUM") as cpsum, \
             tc.tile_pool(name="qpool", bufs=2) as qpool, \
             tc.tile_pool(name="mpool", bufs=1) as mpool:


            for sq in range(30):
                nc.gpsimd.dma_start(w1a[:, 10 * sq:10 * sq + 10, :],
                                    w1a_d[:, 10 * sq:10 * sq + 10, :])
            nc.gpsimd.dma_start(w1b[:], w1b_d[:])
            nc.gpsimd.dma_start(w2s[:], w2_d[:])
            nc.gpsimd.dma_start(woute[:], wout_d[:])
            nc.gpsimd.dma_start(Emat[:], E_d[:])
            nc.gpsimd.dma_start(bout[:], bout_d[:])

            def conv_pool(lhs7, m, Q, C, qi):
                t3 = rpsum.tile([128, 3, 512], f32, tag="t3")
                for j in range(3):
                    nc.tensor.matmul(t3[0:m, j, 0:U], lhs7[j], wc[:],
                                     start=True, stop=True)
                t4 = cpsum.tile([128, 4, 512], f32, tag="t4")
                for j in range(3, 7):
                    nc.tensor.matmul(t4[0:m, j - 3, 0:U], lhs7[j], wc[:],
                                     start=True, stop=True)
                nc.vector.reduce_max(
                    Q[0:m, qi, :], t3[0:m, :, 0:U].rearrange("p j u -> p u j"),
                    axis=mybir.AxisListType.X)
                nc.scalar.activation(C[0:m, qi, :, :], t4[0:m, 0:4, 0:U], AF.Copy)

            def merges(Q, C, m, s0):
                m2 = mpool.tile([128, 4, 2, 300], f16, tag="m2")
                nc.vector.tensor_max(m2[0:m, :, :, :], C[0:m, :, 0:2, :],
                                     C[0:m, :, 2:4, :])
                m3 = mpool.tile([128, 4, 300], f16, tag="m3")
                nc.vector.tensor_max(m3[0:m, :, :], m2[0:m, :, 0, :],
                                     m2[0:m, :, 1, :])
                nc.vector.tensor_max(pm[0:m, s0:s0 + 4, :], m3[0:m, :, :],
                                     Q[0:m, :, :])

            batches = []     # (Q, C, m, pm-slot, exp destination)
            done_m, done_e = 0, 0

            def drain(upto_m, upto_e):
                nonlocal done_m, done_e
                while done_m < upto_m:
                    Q, C, m, s0, _ = batches[done_m]
                    merges(Q, C, m, s0)
                    done_m += 1
                while done_e < upto_e:
                    _, _, m, s0, dst = batches[done_e]
                    nc.scalar.activation(dst[:, :, 0:150], pm[0:m, s0:s0 + 4, 0:150], AF.Exp)
                    nc.scalar.activation(dst[:, :, 150:U], pm[0:m, s0:s0 + 4, 150:U], AF.Exp)
                    done_e += 1

            # B blocks first so their repack DMAs hide under the A rows
            slab0 = xpool.tile([KD, 4, 980], f16, tag="slab")
            nc.sync.dma_start(slab0[:, 0:2, :], xcol_d[:, 0:2, :])
            nc.sync.dma_start(slab0[:, 2:4, :], xcol_d[:, 2:4, :])
            for bq in range(1, 4):
                nc.sync.dma_start(xcolb[:, bq, :, :], xcolb_d[:, bq, :, :])
            Q = qpool.tile([128, 4, 300], f16, tag="Q")
            C = qpool.tile([128, 4, 4, 300], f16, tag="C")
            for blk in range(4):
                lhs7 = [xcolb[:, blk, j, :] for j in range(7)]
                conv_pool(lhs7, 8 * PB, Q, C, blk)
            batches.append((Q, C, 8 * PB, 32, pexpB[0:8 * PB, 0:4, :]))
            for sb in range(8):
                if sb == 0:
                    slab = slab0
                else:
                    slab = xpool.tile([KD, 4, 980], f16, tag="slab")
                    nc.sync.dma_start(slab[:], xcol_d[:, 4 * sb:4 * sb + 4, :])
                Q = qpool.tile([128, 4, 300], f16, tag="Q")
                C = qpool.tile([128, 4, 4, 300], f16, tag="C")
                slabr = slab[:].rearrange("q r (p j) -> q r p j", j=7)
                for r in range(4):
                    lhs7 = [slabr[:, r, 0:PA, j] for j in range(7)]
                    conv_pool(lhs7, PA, Q, C, r)
                    if r == 3:
                        drain(sb + 1, sb + 1)
                if sb == 1:
                    # repack pexpB[(rr,pb), blk, u] -> pexpB2[pb, 8*blk+rr, u]
                    for rr in range(8):
                        nc.sync.dma_start(pexpB2[0:PB, rr:BS:8, :],
                                          pexpB[rr * PB:(rr + 1) * PB, :, :])
                    for off in (32, 64):
                        nc.sync.dma_start(pexpB2[off:off + PB, :, :],
                                          pexpB2[0:PB, :, :])
                batches.append((Q, C, PA, 4 * sb,
                                pexp[0:PA, 4 * sb:4 * sb + 4, :]))
            drain(8, 8)
            # final batch: merge + exp split into unit halves so the left
            # half of pexp (and FC1's first banks) releases mid-chain
            Qf, Cf, mf, s0f, dstf = batches[8]
            for h0, h1 in ((0, 150), (150, 300)):
                m2f = mpool.tile([128, 4, 2, 300], f16, tag="m2")
                nc.vector.tensor_max(m2f[0:mf, :, :, h0:h1], Cf[0:mf, :, 0:2, h0:h1],
                                     Cf[0:mf, :, 2:4, h0:h1])
                m3f = mpool.tile([128, 4, 300], f16, tag="m3")
                nc.vector.tensor_max(m3f[0:mf, :, h0:h1], m2f[0:mf, :, 0, h0:h1],
                                     m2f[0:mf, :, 1, h0:h1])
                nc.vector.tensor_max(pm[0:mf, s0f:s0f + 4, h0:h1],
                                     m3f[0:mf, :, h0:h1], Qf[0:mf, :, h0:h1])
                nc.scalar.activation(dstf[:, :, h0:h1],
                                     pm[0:mf, s0f:s0f + 4, h0:h1], AF.Exp)

        # ---------------- FC1 + relu + FC2 + head ---------------------------
        fcpool = ctx.enter_context(tc.tile_pool(name="fcsb", bufs=1))
        hrelu = fcpool.tile([FC + 1, U, BS], f16)   # [f+const, u, b]
        nc.sync.dma_start(hrelu[FC:FC + 1, :, :].rearrange("p u b -> p (u b)"),
                            ones_d[:])
        zps_pool = ctx.enter_context(tc.tile_pool(name="zpsp", bufs=1, space="PSUM"))
        zps = zps_pool.tile([128, U], f32)

        with tc.tile_pool(name="fcps", bufs=3, space="PSUM") as fpsum:
            for ci, (c0, w) in enumerate(CH):
                for g in range((w + 15) // 16):
                    u0 = c0 + 16 * g
                    nun = min(16, c0 + w - u0)
                    hps = fpsum.tile([FC, 16, BS], f32, tag="hps")
                    for s in range(nun):
                        u = u0 + s
                        o = hps[0:FC, s, 0:BS]
                        nc.tensor.matmul(o, w1a[:, u, :], pexp[:, :, u],
                                         start=True, stop=False)
                        off = 32 * (u % 3)
                        nc.tensor.matmul(
                            o, w1b[off:off + PB, u // 3, :],
                            pexpB2[off:off + PB, :, u], start=False, stop=True)
                    if g % 2:
                        nc.vector.tensor_scalar_max(hrelu[0:FC, u0:u0 + nun, :],
                                                    hps[0:FC, 0:nun, 0:BS], 0.0)
                    else:
                        nc.scalar.activation(hrelu[0:FC, u0:u0 + nun, :],
                                             hps[0:FC, 0:nun, 0:BS], AF.Relu)
                    for k in range(u0 // 4, (u0 + nun) // 4):
                        nc.tensor.matmul(
                            zps[0:128, 4 * k:4 * k + 4],
                            hrelu[0:FC + 1, 4 * k:4 * k + 4, :].rearrange(
                                "f u b -> f (u b)"),
                            w2s[:, k, :], start=True, stop=True)

            # head: full-width relu/mul/reduce; woute zero-masks the
            # off-diagonal strip lanes so they vanish in the reduce
            part = fcpool.tile([128, 1], f32)
            zr = fcpool.tile([128, U], f32)
            prod = fcpool.tile([128, U], f32)
            nc.vector.tensor_scalar_max(zr[:], zps[:], 0.0)
            nc.vector.tensor_mul(prod[:], zr[:], woute[:])
            nc.vector.tensor_reduce(part[:, 0:1], prod[:],
                                    axis=mybir.AxisListType.X, op=ALU.add)
            with tc.tile_pool(name="headps", bufs=1, space="PSUM") as hpsum:
                zf = hpsum.tile([1, BS], f32, tag="zf")
                nc.tensor.matmul(zf[0:1, :], part[:], Emat[:], start=True, stop=True)
                osb = fcpool.tile([1, BS], f32)
                nc.scalar.activation(osb[:], zf[0:1, :], AF.Sigmoid, bias=bout[0:1, :])
                nc.sync.dma_start(out_d[:], osb[:])

    nc.compile()
    return nc


def _prep_weights(i):
    """Host-side BN folding + layout. numpy fp32 math -> fp16 payloads."""
    f = lambda a: np.asarray(a, np.float32)
    w_conv, b_conv = f(i["w_conv"]), f(i["b_conv"])
    g1, be1, m1, v1 = f(i["g1"]), f(i["be1"]), f(i["m1"]), f(i["v1"])
    w_fc1, b_fc1 = f(i["w_fc1"]), f(i["b_fc1"])
    g2, be2, m2, v2 = f(i["g2"]), f(i["be2"]), f(i["m2"]), f(i["v2"])
    w_fc2, b_fc2 = f(i["w_fc2"]), f(i["b_fc2"])
    g3, be3, m3, v3 = f(i["g3"]), f(i["be3"]), f(i["m3"]), f(i["v3"])
    w_out, b_out = f(i["w_out"]), f(i["b_out"])

    s1 = g1 / np.sqrt(v1 + EPS)
    t1 = be1 - m1 * s1
    s2 = g2 / np.sqrt(v2 + EPS)
    b1pp = (b_fc1 - m2) * s2 + be2
    s3 = g3 / np.sqrt(v3 + EPS)
    w2pp = w_fc2 * s3[:, None]
    b2pp = (b_fc2 - m3) * s3 + be3

    # conv weights with BN1 scale folded; contraction index q = k*D + d
    Wc = np.ascontiguousarray(
        (w_conv * s1[:, None, None]).transpose(2, 1, 0).reshape(KD, U))
    # FC1 with BN2 scale and exp(t1 + s1*b_conv) folded
    gexp = np.exp(t1 + s1 * b_conv)
    w1pp = (w_fc1 * s2[:, :, None] * gexp[:, None, None]).transpose(2, 0, 1)  # (P,U,FC)
    w1a = np.empty((128, U, FC), np.float32)
    w1a[:PA] = w1pp[:PA]
    w1a[127] = b1pp                      # bias rides the const-1 pexp row
    # w1b: 3 units per partition group at offsets 0/32/64: [32*(u%3)+pb, u//3, f]
    w1b = np.zeros((64 + PB, FC, FC), np.float32)
    for u in range(U):
        w1b[32 * (u % 3):32 * (u % 3) + PB, u // 3] = w1pp[PA:P, u]

    # FC2 weights f-major with bias row: w2s[f, k, n] = w2pp[4k+n, f]
    w2s = np.empty((FC + 1, NG, 4), np.float32)
    w2s[:FC] = w2pp.T.reshape(FC, NG, 4)
    w2s[FC] = b2pp.reshape(NG, 4)

    # head: full-width, zero-masked: strip n rows keep only cols n::4
    woute = np.zeros((128, U), np.float32)
    for n in range(4):
        woute[32 * n:32 * n + 32, n::4] = w_out[n::4, 0][None]
    Em = np.zeros((128, BS), np.float32)
    for n in range(4):
        Em[32 * n:32 * n + 32] = np.eye(BS, dtype=np.float32)

    h16 = lambda a: np.asarray(a, np.float16)
    return {
        "wc": h16(Wc), "w1a": h16(w1a), "w1b": h16(w1b), "w2s": h16(w2s),
        "woute": woute, "Emat": Em,
        "onesrow": np.ones((1, BS * U), np.float16),
        "bout": np.asarray(b_out, np.float32).reshape(1, 1),
    }


def kernel(**inputs) -> np.ndarray:
    global _COMPILED
    if _COMPILED is None:
        _COMPILED = _build()
    nc = _COMPILED

    wmap = _prep_weights(inputs)
    x = np.asarray(inputs["input_seq"], np.float32)   # (256, 1000, 4)
    win = np.lib.stride_tricks.sliding_window_view(x, K, axis=1)  # (B, 982, D, K)
    in_maps = []
    for c in range(NCORES):
        xs = win[c * BS:(c + 1) * BS, :980]           # (32, 980, 4, 19)
        xcol = np.ascontiguousarray(
            xs.transpose(3, 2, 0, 1).astype(np.float16)).reshape(KD, BS, 980)
        tail = xcol[:, :, 7 * PA:].reshape(KD, 4, 8, PB, 7)
        xcolb = np.ascontiguousarray(tail.transpose(0, 1, 4, 2, 3)).reshape(KD, 4, 7, 104)
        in_maps.append({"xcol": xcol, "xcolb": xcolb, **wmap})

    res = run_bass_kernel_spmd(nc, in_maps, list(range(NCORES)))
    out = np.empty((B, 1), np.float32)
    for c in range(NCORES):
        out[c * BS:(c + 1) * BS, 0] = res.results[c]["out"][0]
    return out
